# revision 28
# baseline (speedup 1.0000x reference)
"""Self-contained Trainium2 kernel for the SSD-scan actor network.

Data-parallel over batch B=8 across 8 NeuronCores (one sample per core, no
collectives). Per core:
  x  = relu(obs @ W_in + b_in)                  (T=512, D=2048)
  dt = softplus(x @ W_dt + dt_bias)             (T, H=16)
  Bm = x @ W_B, Cm = x @ W_C                    (T, H, N=64)
  y  = selective scan over T (Mamba2 SSD)       (T, D)
  z  = relu(y @ W_yo + b_yo)                    (T, U=256)
  out = z @ W_head + b_head                     (T, A=64)

The scan uses the chunked (segsum) SSD formulation: chunk length L=128,
4 chunks, 16 independent heads. Per head/chunk:
  E[j,i]  = exp(Pcum_i - Pcum_j + log dt_j), causally masked to j<=i
  Y^T     = x_chunk^T Gw + S_prev^T (C*u),  Gw = (B C^T)^T . E, u_i=exp(Pcum_i)
  S_new   = exp(Pcum_L-1) S_prev + sum_j exp(Pcum_L-1 - Pcum_j) dt_j B_j x_j^T
Big matmuls run in bf16 with fp32 PSUM accumulation; the Diff matrix
(Pcum_i - Pcum_j + logdt_j) is built exactly in bf16 hi/lo splits via ONE
K=16 block-diagonal matmul per 4-head group (operands bulk-staged once).

v2 perf restructuring vs the first working version:
  - scan heads regrouped as h = 4*hi + hg so the Diff operands for all 16
    (chunk, group) tiles stage with 16 medium DMAs instead of 64 tiny
    just-in-time DMAs + 32 memsets on the critical path.
  - x (t-major) derived from xT by PE transposes (saves 8k matmul cols).
  - scan emission is software-pipelined: group g's Diff+G matmuls are
    emitted before group g-1's Y/S-update block, so the PE never sits in
    the exp->mask->mul latency chain (this idling used to re-throttle the
    PE clock to 1.2 GHz via HAM for ~half the kernel).
  - W_B/W_C streamed with 3-deep prefetch, loads split across the two
    HWDGE issue engines (sync + scalar).
  - z-projection accumulates per-head during the last scan chunk; SBUF
    reuse: y aliases xT, zT aliases obsT, scan rings carve dead w_in.

Hardware notes (all discovered the hard way on this container's stack):
  - walrus here allows only ONE sync wait per instruction -> _split_multi_waits
  - matmul operands whose APs start at partition 64 crash the exec unit
    (NRT_EXEC_UNIT_UNRECOVERABLE), so every matmul operand is kept at base
    partition 0: B/C/Cw are repacked to 64-partition tensors via bf16
    staging + SBUF-to-SBUF DMA (DMA moves across partitions; DVE cannot).
  - Softplus shares no ACT function table with Exp/Ln -> ln(1+exp(x)).
"""

import sys
import types

import numpy as np
import ml_dtypes

T, BSZ, OBSD = 512, 8, 256
D, H, N, P = 2048, 16, 64, 128
U, A = 256, 64
L, NCH, KT = 128, 4, 16  # chunk length, #chunks, #d-tiles (D/128)
HN = H * N
BF16 = ml_dtypes.bfloat16

_CACHE = {}
_EXECUTED = {}


def _patch_tile():
    """Split the TileContext final drain's waits across single-wait nops."""
    from concourse import tile, mybir
    from concourse.vector_clock import ScopedClock

    if getattr(tile.TileContext, "_drain_patched", False):
        return

    def _patched(self, tick_clock, wait_clock):
        nc = self.nc
        probe = nc.sync.nop()
        wait_clock.add_sem_waits(
            probe.ins, ScopedClock({None: tick_clock.global_clock})
        )
        si = probe.ins.sync_info
        if si is not None and len(si.on_wait) > 1:
            waits = list(si.on_wait)
            probe.ins.sync_info = mybir.SyncInfo(
                on_wait=[waits[0]], on_update=list(si.on_update)
            )
            for w in waits[1:]:
                nop = nc.sync.nop()
                nop.ins.sync_info = mybir.SyncInfo(on_wait=[w], on_update=[])
        nc.sync.drain()
        nc.all_engine_barrier(sem_only=True)
        assert self.sems is not None
        popped = nc._tile_sem_poison_stack.pop()
        assert popped is self._sem_poison
        nc.clear_and_free_semaphores(list(self.sems.allocated().values()))
        nc.all_engine_barrier(sem_only=True)

    tile.TileContext._drain_and_barrier = _patched
    tile.TileContext._drain_patched = True


def _split_multi_waits(nc):
    """This walrus build accepts at most one sync wait per instruction.
    Hoist extra waits onto single-wait NoOps inserted just before, on the
    same engine (the sequencer stalls there first — strictly conservative)."""
    from concourse import mybir

    n = 0
    for f in nc.m.functions:
        for bb in f.blocks:
            insts = list(bb.instructions)
            changed = False
            new = []
            for inst in insts:
                try:
                    si = inst.sync_info
                except Exception:
                    si = None
                if si is not None and len(si.on_wait) > 1:
                    waits = list(si.on_wait)
                    for w in waits[:-1]:
                        nop = mybir.InstNoOp(
                            name=f"wsplit-{n}", ins=[], outs=[], engine=inst.engine
                        )
                        n += 1
                        nop.sync_info = mybir.SyncInfo(on_wait=[w], on_update=[])
                        nc.register_instruction(nop, overwrite=True)
                        new.append(nop)
                    inst.sync_info = mybir.SyncInfo(
                        on_wait=[waits[-1]], on_update=list(si.on_update)
                    )
                    changed = True
                new.append(inst)
            if changed:
                bb.instructions = new


def _inject_axon_hooks():
    """Make trace=True work (and a BASS_TRACE env var safe) in this container."""
    if "antenv.axon_hooks" not in sys.modules:
        try:
            from trn_agent_boot.trn_boot import _ntff_profile_via_ctypes

            hook = _ntff_profile_via_ctypes("/opt/axon/libaxon_pjrt.so")
        except Exception:
            hook = None
        mod = types.ModuleType("antenv.axon_hooks")
        mod.get_axon_ntff_profile_hook = lambda: hook
        mod.set_axon_ntff_profile_hook = lambda h: None
        sys.modules["antenv.axon_hooks"] = mod
    from concourse import bass_utils

    bass_utils.upload_artifacts = lambda d: d


def _build(with_b_in, with_b_yo, with_b_head):
    import concourse.bass as bass
    import concourse.mybir as mybir
    from concourse.tile import TileContext
    from concourse.masks import make_identity

    dt = mybir.dt
    AF = mybir.ActivationFunctionType
    OP = mybir.AluOpType

    nc = bass.Bass()
    obsT_e = nc.declare_dram_parameter("obsT", [OBSD, T], dt.bfloat16, isOutput=False)
    w_in_e = nc.declare_dram_parameter("w_in", [OBSD, D], dt.bfloat16, isOutput=False)
    w_dt_e = nc.declare_dram_parameter("w_dt", [D, H], dt.bfloat16, isOutput=False)
    w_b_e = nc.declare_dram_parameter("w_b", [D, HN], dt.bfloat16, isOutput=False)
    w_c_e = nc.declare_dram_parameter("w_c", [D, HN], dt.bfloat16, isOutput=False)
    w_yo_e = nc.declare_dram_parameter("w_yo", [D, U], dt.bfloat16, isOutput=False)
    w_hd_e = nc.declare_dram_parameter("w_hd", [U, A], dt.bfloat16, isOutput=False)
    neg_a_e = nc.declare_dram_parameter("neg_a", [H, 1], dt.float32, isOutput=False)
    dtb_e = nc.declare_dram_parameter("dtb", [H, 1], dt.float32, isOutput=False)
    bin_e = byo_e = bhd_e = None
    if with_b_in:
        bin_e = nc.declare_dram_parameter("b_in", [P, KT], dt.float32, isOutput=False)
    if with_b_yo:
        byo_e = nc.declare_dram_parameter("b_yo", [P, 2], dt.float32, isOutput=False)
    if with_b_head:
        bhd_e = nc.declare_dram_parameter("b_hd", [1, A], dt.bfloat16, isOutput=False)
    out_e = nc.declare_dram_parameter("out", [T, A], dt.float32, isOutput=True)

    _patch_tile()
    with TileContext(nc) as tc:
        with (
            tc.tile_pool(name="cst", bufs=1) as cst,
            tc.tile_pool(name="wrk", bufs=2) as wrk,
            tc.tile_pool(name="ps_proj", bufs=2, space="PSUM") as ps_proj,
            tc.tile_pool(name="ps_diff", bufs=2, space="PSUM") as ps_diff,
            tc.tile_pool(name="ps_gy", bufs=2, space="PSUM") as ps_gy,
            tc.tile_pool(name="ps_sd", bufs=1, space="PSUM") as ps_sd,
            tc.tile_pool(name="ps_tiny", bufs=1, space="PSUM") as ps_tiny,
        ):
            # ---------------- weights + constants -----------------
            obsT = cst.tile([P, 2 * T], dt.bfloat16, tag="obsT")
            nc.sync.dma_start(
                out=obsT[:].rearrange("p (k t) -> p k t", k=2),
                in_=obsT_e.rearrange("(k p) t -> p k t", p=P),
            )
            w_in = cst.tile([P, 2 * D], dt.bfloat16, tag="w_in")
            nc.sync.dma_start(
                out=w_in[:].rearrange("p (k d) -> p k d", k=2),
                in_=w_in_e.rearrange("(k p) d -> p k d", p=P),
            )
            w_dt = cst.tile([P, KT * H], dt.bfloat16, tag="w_dt")
            nc.sync.dma_start(
                out=w_dt[:].rearrange("p (k h) -> p k h", k=KT),
                in_=w_dt_e.rearrange("(k p) h -> p k h", p=P),
            )
            w_yo = cst.tile([P, KT * U], dt.bfloat16, tag="w_yo")
            nc.scalar.dma_start(
                out=w_yo[:].rearrange("p (k u) -> p k u", k=KT),
                in_=w_yo_e.rearrange("(k p) u -> p k u", p=P),
            )
            w_hd = cst.tile([P, 2 * A], dt.bfloat16, tag="w_hd")
            nc.scalar.dma_start(
                out=w_hd[:].rearrange("p (k a) -> p k a", k=2),
                in_=w_hd_e.rearrange("(k p) a -> p k a", p=P),
            )
            neg_a = cst.tile([H, 1], dt.float32, tag="neg_a")
            nc.sync.dma_start(out=neg_a[:], in_=neg_a_e[:])
            dtb = cst.tile([H, 1], dt.float32, tag="dtb")
            nc.sync.dma_start(out=dtb[:], in_=dtb_e[:])
            if with_b_in:
                b_in = cst.tile([P, KT], dt.float32, tag="b_in")
                nc.sync.dma_start(out=b_in[:], in_=bin_e[:])
            if with_b_yo:
                b_yo = cst.tile([P, 2], dt.float32, tag="b_yo")
                nc.sync.dma_start(out=b_yo[:], in_=byo_e[:])
            if with_b_head:
                b_hd = cst.tile([1, A], dt.bfloat16, tag="b_hd")
                nc.sync.dma_start(out=b_hd[:], in_=bhd_e[:])

            ident_f = cst.tile([H, H], dt.float32, tag="ident_f")
            make_identity(nc, ident_f[:])
            ident_pb = cst.tile([P, P], dt.bfloat16, tag="ident_pb")
            make_identity(nc, ident_pb[:])
            # causal ones: UT[j,i] = 1 where j<=i (cumsum matmul)
            ut_ones = cst.tile([L, L], dt.float32, tag="ut_ones")
            nc.gpsimd.memset(ut_ones[:], 1.0)
            nc.gpsimd.affine_select(
                out=ut_ones[:],
                in_=ut_ones[:],
                compare_op=OP.is_ge,
                fill=0.0,
                base=0,
                pattern=[[1, L]],
                channel_multiplier=-1,
            )
            ones_row = cst.tile([1, P], dt.float32, tag="ones_row")
            nc.gpsimd.memset(ones_row[:], 1.0)
            # lh16 row 0 is a constant-1.0 row for its whole life: it doubles
            # as the broadcast-ones source for fills and the b_head matmul.
            def get_ones2k():
                return lh16
            # head-pair row selector: ubp = sel2^T @ urow replicates u_he to
            # partitions 0..63 and u_ho to 64..127 in ONE matmul.  Built with
            # two affine_selects (memset can't start at partition 1).
            sel2 = cst.tile([2, P], dt.bfloat16, tag="sel2")
            nc.gpsimd.memset(sel2[:], 1.0)
            nc.gpsimd.affine_select(
                out=sel2[:], in_=sel2[:], compare_op=OP.is_ge, fill=0.0,
                base=0, pattern=[[1, P]], channel_multiplier=-N,
            )
            nc.gpsimd.affine_select(
                out=sel2[:], in_=sel2[:], compare_op=OP.is_ge, fill=0.0,
                base=N - 1, pattern=[[-1, P]], channel_multiplier=N,
            )

            # ---------------- activations / state ------------------
            xT = cst.tile([P, KT * T], dt.bfloat16, tag="xT")  # (d, t); later = y
            x = cst.tile([P, NCH * D], dt.bfloat16, tag="x")  # (t, d) per t-tile
            # B/C/Cw repacked: 64 partitions, head h at columns [h*T,(h+1)*T)
            bm = cst.tile([N, H * T], dt.bfloat16, tag="bm")
            cm = cst.tile([N, H * T], dt.bfloat16, tag="cm")
            cw = cst.tile([N, H * T], dt.bfloat16, tag="cw")
            y = cst.tile([P, KT * T], dt.bfloat16, tag="y")  # (d, t) per d-tile
            zT = obsT  # reuse: obs consumed by phase 1
            logit = cst.tile([P, NCH * A], dt.float32, tag="logit")
            s_st = [
                cst.tile([N, P], dt.bfloat16, tag=f"s{h}", name=f"s{h}")
                for h in range(H)
            ]
            for h in range(H):
                nc.gpsimd.memset(s_st[h][:], 0.0)

            dtT = cst.tile([H, T], dt.float32, tag="dtT")
            logdtT = cst.tile([H, T], dt.float32, tag="logdtT")
            pcumT = cst.tile([H, T], dt.float32, tag="pcumT")  # Pcum rows (h,t)
            # two-float (hi/lo bf16) splits for the K=16 bf16 Diff matmul
            pcumH = cst.tile([H, T], dt.bfloat16, tag="pcumH")
            pcumL = cst.tile([H, T], dt.bfloat16, tag="pcumL")
            npdH = cst.tile([H, T], dt.bfloat16, tag="npdH")
            npdL = cst.tile([H, T], dt.bfloat16, tag="npdL")
            plrow = cst.tile([1, NCH * H], dt.float32, tag="plrow")
            u_all = cst.tile([H, T], dt.bfloat16, tag="u_all")  # exp(Pcum)
            cols = cst.tile([P, NCH * 2 * H], dt.float32, tag="cols")  # [PcumCol|dtCol]
            e2c = cst.tile([P, NCH * H], dt.float32, tag="e2c")  # exp(Plast-Pcum)
            dtotc = cst.tile([P, NCH * H], dt.float32, tag="dtotc")
            ulast = cst.tile([1, NCH * H], dt.float32, tag="ulast")
            # K=16 block-diag Diff operands, all 16 (c,hg) groups staged once.
            # Group (c,hg) holds heads h=4*hi+hg; lh16 col block (hg,c) of 128,
            # rp16 col block (hg,c) of 512 (4 heads x L, diag-block layout).
            lh16 = cst.tile([H, 16 * L], dt.bfloat16, tag="lh16")
            rp16 = cst.tile([H, 16 * 4 * L], dt.bfloat16, tag="rp16")
            nc.gpsimd.memset(lh16[:], 1.0)
            nc.gpsimd.memset(rp16[:], 0.0)
            # constant-1 rows of rp16 (rows 4hi+1, 4hi+3, diag block hi only);
            # DMA-filled: memset can't address a nonzero base partition.
            for hi in range(4):
                for r in (1, 3):
                    dst = rp16[4 * hi + r : 4 * hi + r + 1, :].rearrange(
                        "p (g c i t) -> p g c i t", g=4, c=NCH, i=4
                    )[:, :, :, hi : hi + 1, :]
                    eng = nc.sync if r == 1 else nc.scalar
                    eng.dma_start(
                        out=dst,
                        in_=get_ones2k()[0:1, :].rearrange(
                            "p (g c t) -> p g c t", g=4, c=NCH
                        ),
                    )

            # ---------------- x^T = relu(W_in^T obs^T) (d,t) --------
            for kt in range(KT):
                ps = ps_proj.tile([P, T], dt.float32, tag="proj")
                for ko in range(2):
                    nc.tensor.matmul(
                        ps[:],
                        w_in[:, ko * D + kt * P : ko * D + (kt + 1) * P],
                        obsT[:, ko * T : (ko + 1) * T],
                        start=(ko == 0),
                        stop=(ko == 1),
                    )
                if with_b_in:
                    nc.scalar.activation(
                        xT[:, kt * T : (kt + 1) * T], ps[:], AF.Relu,
                        bias=b_in[:, kt : kt + 1],
                    )
                else:
                    nc.scalar.activation(xT[:, kt * T : (kt + 1) * T], ps[:], AF.Relu)

            # ---------------- x = xT^T via PE transposes (t,d) ------
            for tt in range(NCH):
                for g4 in range(4):
                    ps = ps_proj.tile([P, 4 * P], dt.bfloat16, tag="proj")
                    for kk in range(4):
                        kt = g4 * 4 + kk
                        nc.tensor.transpose(
                            ps[:, kk * P : (kk + 1) * P],
                            xT[:, kt * T + tt * P : kt * T + (tt + 1) * P],
                            ident_pb[:],
                        )
                    nc.vector.tensor_copy(
                        x[:, tt * D + g4 * 512 : tt * D + (g4 + 1) * 512], ps[:]
                    )

            # ---------------- dt chain ------------------------------
            psd = ps_proj.tile([H, T], dt.float32, tag="proj", name="psd")
            for kt in range(KT):
                nc.tensor.matmul(
                    psd[:],
                    w_dt[:, kt * H : (kt + 1) * H],
                    xT[:, kt * T : (kt + 1) * T],
                    start=(kt == 0),
                    stop=(kt == KT - 1),
                )
            # softplus via ln(1+exp(.)) — Softplus shares no ACT table with
            # Exp/Ln on this compiler; exp/ln/relu/copy live in one table.
            ez = ps_diff.tile([H, T], dt.float32, tag="diff", name="ez")
            nc.scalar.activation(ez[:], psd[:], AF.Exp, bias=dtb[:])
            nc.vector.tensor_scalar_add(ez[:], ez[:], 1.0)
            nc.scalar.activation(dtT[:], ez[:], AF.Ln)
            nc.scalar.activation(logdtT[:], dtT[:], AF.Ln)

            for c in range(NCH):
                cb = slice(c * L, (c + 1) * L)
                ldec = wrk.tile([H, L], dt.float32, tag="ldec", bufs=1)
                nc.vector.tensor_scalar_mul(ldec[:], dtT[:, cb], neg_a[:])
                pt = ps_tiny.tile([P, 2 * H], dt.float32, tag="tiny")
                nc.tensor.transpose(pt[:, 0:H], ldec[:], ident_f[0:H, 0:H])
                ldec_c = wrk.tile([P, H], dt.float32, tag="ldec_c")
                nc.vector.tensor_copy(ldec_c[:], pt[:, 0:H])
                pp = ps_tiny.tile([H, L], dt.float32, tag="tiny")
                nc.tensor.matmul(pp[:], ldec_c[:], ut_ones[:], start=True, stop=True)
                nc.vector.tensor_copy(pcumT[:, cb], pp[:])
                npdc = wrk.tile([H, L], dt.float32, tag="npdc")
                nc.vector.tensor_sub(npdc[:], logdtT[:, cb], pcumT[:, cb])
                nc.vector.tensor_copy(pcumH[:, cb], pcumT[:, cb])
                nc.vector.tensor_sub(pcumL[:, cb], pcumT[:, cb], pcumH[:, cb])
                nc.vector.tensor_copy(npdH[:, cb], npdc[:])
                nc.vector.tensor_sub(npdL[:, cb], npdc[:], npdH[:, cb])
                nc.scalar.activation(u_all[:, cb], pcumT[:, cb], AF.Exp)
                pt2 = ps_tiny.tile([P, 2 * H], dt.float32, tag="tiny")
                nc.tensor.transpose(pt2[:, 0:H], pcumT[:, cb], ident_f[0:H, 0:H])
                nc.tensor.transpose(pt2[:, H : 2 * H], dtT[:, cb], ident_f[0:H, 0:H])
                co = c * 2 * H
                nc.vector.tensor_copy(cols[:, co : co + 2 * H], pt2[:])
                # PcumLast per head at base partition 0 (row 127 of PcumCol)
                nc.sync.dma_start(
                    out=plrow[:, c * H : (c + 1) * H],
                    in_=cols[L - 1 : L, co : co + H],
                )
                plast = plrow[:, c * H : (c + 1) * H]
                nc.scalar.activation(ulast[:, c * H : (c + 1) * H], plast, AF.Exp)
                pdt = ps_tiny.tile([P, H], dt.float32, tag="tiny")
                nc.tensor.matmul(
                    pdt[:],
                    ones_row[0:1, 0:P],
                    ulast[:, c * H : (c + 1) * H],
                    start=True,
                    stop=True,
                )
                nc.vector.tensor_copy(dtotc[:, c * H : (c + 1) * H], pdt[:])
                ppl = ps_tiny.tile([P, H], dt.float32, tag="tiny")
                nc.tensor.matmul(
                    ppl[:], ones_row[0:1, 0:P], plast, start=True, stop=True
                )
                e2a = wrk.tile([P, H], dt.float32, tag="e2a")
                nc.vector.tensor_sub(e2a[:], ppl[:], cols[:, co : co + H])
                nc.scalar.activation(e2c[:, c * H : (c + 1) * H], e2a[:], AF.Exp)

            # bulk-stage the Diff operand variable rows (all chunks at once):
            # rp16 rows {4hi+0, 4hi+2} <- pcum hi/lo of heads {4hi..4hi+3};
            # lh16 rows {4hi+1, 4hi+3} <- npd hi/lo.  One DMA per head-quad:
            # dst iter (s=hi/lo, g=hg, c, t) == src iter (s, h, c, t).
            for hi in range(4):
                for r, src in ((0, pcumH), (2, pcumL)):
                    dst = rp16[4 * hi + r : 4 * hi + r + 1, :].rearrange(
                        "p (g c i t) -> p g c i t", g=4, c=NCH, i=4
                    )[:, :, :, hi : hi + 1, :]
                    eng = nc.sync if r == 0 else nc.scalar
                    eng.dma_start(
                        out=dst,
                        in_=src[4 * hi : 4 * hi + 4, :].rearrange(
                            "h (c t) -> h c t", c=NCH
                        ),
                    )
                for r, src in ((1, npdH), (3, npdL)):
                    dst = lh16[4 * hi + r : 4 * hi + r + 1, :].rearrange(
                        "p (g c j) -> p g c j", g=4, c=NCH
                    )
                    eng = nc.sync if r == 1 else nc.scalar
                    eng.dma_start(
                        out=dst,
                        in_=src[4 * hi : 4 * hi + 4, :].rearrange(
                            "h (c j) -> h c j", c=NCH
                        ),
                    )

            # ---------------- B / C projections ---------------------
            # psum (128, T) holds heads (2mt, 2mt+1); evacuate to bf16 staging
            # then DMA each 64-partition half into the base-0 packed tensors.
            # Weight loads are paired (2 mts per DMA); even mts project before
            # the scan, odd mts interleave INTO the scan's first half so the
            # PE stays dense (HAM un-throttled).
            bmt = cst.tile([P, H * NCH * N], dt.bfloat16, tag="bmt")  # B^T (t,n)
            # host side permutes head-pair blocks to [0,2,4,6,1,3,5,7] so each
            # paired (2-mt) load is one contiguous 256-col slice (3-dim AP)
            wbv = w_b_e.rearrange("(kt p) (j m) -> p kt j m", p=P, j=4)
            wcv = w_c_e.rearrange("(kt p) (j m) -> p kt j m", p=P, j=4)

            def emit_wload(j):
                wbuf2 = wrk.tile([P, KT * 2 * P], dt.bfloat16, tag="wbs", bufs=2)
                nc.sync.dma_start(
                    out=wbuf2[:].rearrange("p (kt m) -> p kt m", kt=KT),
                    in_=wbv[:, :, j, :],
                )
                wcuf2 = wrk.tile([P, KT * 2 * P], dt.bfloat16, tag="wcs", bufs=2)
                nc.scalar.dma_start(
                    out=wcuf2[:].rearrange("p (kt m) -> p kt m", kt=KT),
                    in_=wcv[:, :, j, :],
                )
                return wbuf2, wcuf2

            def emit_proj(mt, u, wbuf2, wcuf2):
                he, ho = 2 * mt, 2 * mt + 1
                psb = ps_proj.tile([P, T], dt.float32, tag="proj")
                for kt in range(KT):
                    nc.tensor.matmul(
                        psb[:],
                        wbuf2[:, kt * 2 * P + u * P : kt * 2 * P + (u + 1) * P],
                        xT[:, kt * T : (kt + 1) * T],
                        start=(kt == 0),
                        stop=(kt == KT - 1),
                    )
                btmp = wrk.tile([P, T], dt.bfloat16, tag="btmp", bufs=2)
                nc.vector.tensor_copy(btmp[:], psb[:])
                nc.sync.dma_start(out=bm[:, he * T : (he + 1) * T], in_=btmp[0:N, :])
                nc.sync.dma_start(out=bm[:, ho * T : (ho + 1) * T], in_=btmp[N:P, :])
                psc = ps_proj.tile([P, T], dt.float32, tag="proj")
                for kt in range(KT):
                    nc.tensor.matmul(
                        psc[:],
                        wcuf2[:, kt * 2 * P + u * P : kt * 2 * P + (u + 1) * P],
                        xT[:, kt * T : (kt + 1) * T],
                        start=(kt == 0),
                        stop=(kt == KT - 1),
                    )
                ctmp = wrk.tile([P, T], dt.bfloat16, tag="ctmp", bufs=2)
                nc.vector.tensor_copy(ctmp[:], psc[:])
                nc.sync.dma_start(out=cm[:, he * T : (he + 1) * T], in_=ctmp[0:N, :])
                nc.sync.dma_start(out=cm[:, ho * T : (ho + 1) * T], in_=ctmp[N:P, :])
                # u-scaled C: one (2,T) stage DMA + one sel2-matmul broadcast
                urow = wrk.tile([2, T], dt.bfloat16, tag="urow", bufs=1)
                nc.scalar.dma_start(out=urow[:], in_=u_all[he : ho + 1, :])
                ubp = ps_diff.tile([P, 4 * L], dt.float32, tag="diff", name="ubp")
                nc.tensor.matmul(ubp[:], sel2[:], urow[:], start=True, stop=True)
                ubc = wrk.tile([P, T], dt.bfloat16, tag="ubc", bufs=1)
                nc.scalar.activation(ubc[:], ubp[:], AF.Copy)
                wtmp = wrk.tile([P, T], dt.bfloat16, tag="wtmp", bufs=2)
                nc.vector.tensor_mul(wtmp[:], psc[:], ubc[:])
                nc.scalar.dma_start(out=cw[:, he * T : (he + 1) * T], in_=wtmp[0:N, :])
                nc.scalar.dma_start(out=cw[:, ho * T : (ho + 1) * T], in_=wtmp[N:P, :])
                # B^T (t,n) blocks for the S update, transposed off the scan path
                ptr = ps_tiny.tile([P, 4 * P], dt.bfloat16, tag="tiny")
                for q, hh in enumerate((he, ho)):
                    for cc in range(NCH):
                        nc.tensor.transpose(
                            ptr[:, q * NCH * N + cc * N : q * NCH * N + (cc + 1) * N],
                            bm[:, hh * T + cc * L : hh * T + (cc + 1) * L],
                            ident_pb[0:N, 0:N],
                        )
                nc.vector.tensor_copy(
                    bmt[:, he * NCH * N : (ho + 1) * NCH * N], ptr[:]
                )

            wpair = {}
            wpair[0] = emit_wload(0)  # mts 0,2
            wpair[1] = emit_wload(1)  # mts 4,6
            emit_proj(0, 0, *wpair[0])
            emit_proj(2, 1, *wpair[0])
            emit_proj(4, 0, *wpair[1])
            emit_proj(6, 1, *wpair[1])
            wpair[2] = emit_wload(2)  # mts 1,3
            wpair[3] = emit_wload(3)  # mts 5,7

            # ---------------- scan ----------------------------------
            # y cols (h, t) with h = 4*hi + hg: strided evac view per group.
            yv4 = y[:].rearrange("p (i g t) -> p i g t", i=4, g=4)
            # e_sb / gw / bd rings carved from the dead w_in tile (bf16,
            # (P, 4096) = 8 x 512-col slots).  w_in's last read is phase 1.
            esb_ring = [w_in[:, i * 512 : (i + 1) * 512] for i in range(3)]
            gw_ring = [w_in[:, (3 + i) * 512 : (4 + i) * 512] for i in range(3)]
            # chunk-major: consecutive groups touch different heads, so the
            # serial per-head state chain is 2 groups apart and the scan
            # pipelines without waiting on the S-update chain.  Emission is
            # software-pipelined: group g's Diff+G matmuls go to the PE before
            # group g-1's Y/S block, hiding the exp->mask->mul latency.
            st = {"pend": None, "g": 0, "z": 0}

            def z_head(h):
                # both u-halves of the z accumulation for one finished head
                for ut in range(2):
                    nc.tensor.matmul(
                        z_ps[ut][:],
                        w_yo[:, h * U + ut * P : h * U + (ut + 1) * P],
                        y[:, h * T : (h + 1) * T],
                        start=(st["z"] == 0),
                        stop=(st["z"] == H - 1),
                    )
                st["z"] += 1

            def group_front(c, hg):
                grp = hg * 4 + c
                dbank = ps_diff.tile([P, 4 * L], dt.float32, tag="diff")
                nc.tensor.matmul(
                    dbank[:],
                    lh16[:, grp * L : (grp + 1) * L],
                    rp16[:, grp * 4 * L : (grp + 1) * 4 * L],
                    start=True,
                    stop=True,
                )
                gbank = ps_gy.tile([P, 4 * L], dt.float32, tag="gy")
                for hi in range(4):
                    h = 4 * hi + hg
                    hb = slice(h * T + c * L, h * T + (c + 1) * L)
                    nc.tensor.matmul(
                        gbank[:, hi * L : (hi + 1) * L],
                        bm[:, hb],
                        cm[:, hb],
                        start=True,
                        stop=True,
                    )
                gi = st["g"]
                e_sb = esb_ring[gi % 3]
                nc.scalar.activation(e_sb, dbank[:], AF.Exp)
                # causal mask: keep i>=j else 0 (kills the exp-overflow infs)
                nc.gpsimd.affine_select(
                    out=e_sb,
                    in_=e_sb,
                    compare_op=OP.is_ge,
                    fill=0.0,
                    base=0,
                    pattern=[[0, 4], [1, L]],
                    channel_multiplier=-1,
                )
                gw = gw_ring[gi % 3]
                nc.vector.tensor_mul(gw, gbank[:], e_sb)
                if st["pend"] is not None:
                    st["pend"]()

                def consume(c=c, hg=hg, gw=gw):
                    ybank = ps_gy.tile([P, 4 * L], dt.float32, tag="gy")
                    sdb = ps_sd.tile([N, 4 * P], dt.float32, tag="sd")
                    bd = wrk.tile([P, 4 * N], dt.bfloat16, tag="bd")
                    for hi in range(4):
                        h = 4 * hi + hg
                        hb = slice(h * T + c * L, h * T + (c + 1) * L)
                        xc = x[:, c * D + h * P : c * D + (h + 1) * P]
                        nc.tensor.matmul(
                            ybank[:, hi * L : (hi + 1) * L],
                            xc,
                            gw[:, hi * L : (hi + 1) * L],
                            start=True,
                            stop=False,
                        )
                        nc.tensor.matmul(
                            ybank[:, hi * L : (hi + 1) * L],
                            s_st[h][:],
                            cw[:, hb],
                            start=False,
                            stop=True,
                        )
                        nc.vector.tensor_scalar(
                            bd[:, hi * N : (hi + 1) * N],
                            bmt[:, h * NCH * N + c * N : h * NCH * N + (c + 1) * N],
                            e2c[:, c * H + h : c * H + h + 1],
                            cols[:, c * 2 * H + H + h : c * 2 * H + H + h + 1],
                            op0=OP.mult,
                            op1=OP.mult,
                        )
                        sds = sdb[:, hi * P : (hi + 1) * P]
                        nc.tensor.matmul(
                            sds, bd[:, hi * N : (hi + 1) * N], xc,
                            start=True, stop=True,
                        )
                        nc.vector.scalar_tensor_tensor(
                            s_st[h][:],
                            s_st[h][:],
                            dtotc[0:N, c * H + h : c * H + h + 1],
                            sds,
                            op0=OP.mult,
                            op1=OP.add,
                        )
                    # Y evac: psum (p, (hi,L)) -> y cols (4*hi+hg, c*L..)
                    nc.scalar.activation(
                        yv4[:, :, hg : hg + 1, c * L : (c + 1) * L],
                        ybank[:].rearrange("p (i t) -> p i t", i=4),
                        AF.Copy,
                    )
                    # z for this group's heads (the hg>=2 heads finish here;
                    # hg<2 heads' z is interleaved into the second half-block)
                    if c == NCH - 1 and hg >= 2:
                        for hi in range(4):
                            z_head(4 * hi + hg)

                st["pend"] = consume
                st["g"] += 1

            # first half-block: heads with hg in {0,1}; odd-mt projections
            # interleave here to keep the PE dense while the scan chains run.
            g01 = [(c, hg) for c in range(NCH) for hg in (0, 1)]
            for idx, (c, hg) in enumerate(g01):
                group_front(c, hg)
                if idx == 1:
                    emit_proj(1, 0, *wpair[2])
                elif idx == 3:
                    emit_proj(3, 1, *wpair[2])
                elif idx == 5:
                    emit_proj(5, 0, *wpair[3])
                elif idx == 7:
                    emit_proj(7, 1, *wpair[3])
            z_ps = [
                ps_proj.tile([P, T], dt.float32, tag="proj", name=f"z{ut}")
                for ut in range(2)
            ]
            # second half-block: heads with hg in {2,3}; finished hg01 heads'
            # z matmuls interleave here (one head per group).
            zq = [4 * hi + hg for hg in (0, 1) for hi in range(4)]
            g23 = [(c, hg) for c in range(NCH) for hg in (2, 3)]
            for idx, (c, hg) in enumerate(g23):
                group_front(c, hg)
                z_head(zq[idx])
            st["pend"]()

            # ---------------- z = relu(y W_yo) (u,t) ----------------
            for ut in range(2):
                if with_b_yo:
                    nc.scalar.activation(
                        zT[:, ut * T : (ut + 1) * T], z_ps[ut][:], AF.Relu,
                        bias=b_yo[:, ut : ut + 1],
                    )
                else:
                    nc.scalar.activation(
                        zT[:, ut * T : (ut + 1) * T], z_ps[ut][:], AF.Relu
                    )

            # ---------------- logits --------------------------------
            for tt in range(NCH):
                ps = ps_proj.tile([P, A], dt.float32, tag="proj")
                nmm = 3 if with_b_head else 2
                for ut in range(2):
                    nc.tensor.matmul(
                        ps[:],
                        zT[:, ut * T + tt * P : ut * T + (tt + 1) * P],
                        w_hd[:, ut * A : (ut + 1) * A],
                        start=(ut == 0),
                        stop=(ut == nmm - 1),
                    )
                if with_b_head:
                    nc.tensor.matmul(
                        ps[:],
                        get_ones2k()[0:1, tt * P : (tt + 1) * P],
                        b_hd[:],
                        start=False,
                        stop=True,
                    )
                nc.scalar.activation(logit[:, tt * A : (tt + 1) * A], ps[:], AF.Copy)
                nc.sync.dma_start(
                    out=out_e[tt * P : (tt + 1) * P, :],
                    in_=logit[:, tt * A : (tt + 1) * A],
                )

    _split_multi_waits(nc)
    return nc


def kernel(obs, W_in, b_in, A_log, dt_bias, W_dt, W_B, W_C, W_yo, b_yo, W_head, b_head):
    _inject_axon_hooks()
    _patch_tile()
    from concourse.bass_utils import run_bass_kernel_spmd

    obs = np.asarray(obs, dtype=np.float32)
    flags = (
        bool(np.any(np.asarray(b_in) != 0)),
        bool(np.any(np.asarray(b_yo) != 0)),
        bool(np.any(np.asarray(b_head) != 0)),
    )
    # First call: build once (the verified path). Repeat calls in one
    # process rebuild a fresh graph — re-executing a previously-run nc with
    # new inputs has crashed the exec unit (NRT status 101) in testing.
    if flags not in _CACHE:
        _CACHE[flags] = _build(*flags)
    elif _EXECUTED.get(flags):
        _CACHE[flags] = _build(*flags)
    nc = _CACHE[flags]
    _EXECUTED[flags] = True

    obsT = obs.reshape(T, BSZ, OBSD).transpose(1, 2, 0)  # (B, 256, T)
    base = {
        "w_in": np.ascontiguousarray(W_in).astype(BF16),
        "w_dt": np.ascontiguousarray(W_dt).astype(BF16),
        "w_b": np.ascontiguousarray(
            np.asarray(W_B).reshape(D, 8, P)[:, [0, 2, 4, 6, 1, 3, 5, 7], :]
            .reshape(D, HN)
        ).astype(BF16),
        "w_c": np.ascontiguousarray(
            np.asarray(W_C).reshape(D, 8, P)[:, [0, 2, 4, 6, 1, 3, 5, 7], :]
            .reshape(D, HN)
        ).astype(BF16),
        "w_yo": np.ascontiguousarray(W_yo).astype(BF16),
        "w_hd": np.ascontiguousarray(W_head).astype(BF16),
        "neg_a": (-np.exp(np.asarray(A_log, np.float64)))
        .astype(np.float32)
        .reshape(H, 1),
        "dtb": np.asarray(dt_bias, np.float32).reshape(H, 1),
    }
    if flags[0]:
        base["b_in"] = np.ascontiguousarray(
            np.asarray(b_in, np.float32).reshape(KT, P).T
        )
    if flags[1]:
        base["b_yo"] = np.ascontiguousarray(
            np.asarray(b_yo, np.float32).reshape(2, P).T
        )
    if flags[2]:
        base["b_hd"] = np.asarray(b_head).astype(BF16).reshape(1, A)
    in_maps = [
        dict(base, obsT=np.ascontiguousarray(obsT[c]).astype(BF16)) for c in range(BSZ)
    ]
    global _last_in_maps
    _last_in_maps = in_maps
    res = run_bass_kernel_spmd(nc, in_maps, core_ids=list(range(BSZ)))
    out = np.stack([res.results[c]["out"] for c in range(BSZ)], axis=1)
    return out.astype(np.float32)


# revision 29
# speedup vs baseline: 1.1540x; 1.1540x over previous
"""Self-contained Trainium2 kernel for the SSD-scan actor network.

Data-parallel over batch B=8 across 8 NeuronCores (one sample per core, no
collectives). Per core:
  x  = relu(obs @ W_in + b_in)                  (T=512, D=2048)
  dt = softplus(x @ W_dt + dt_bias)             (T, H=16)
  Bm = x @ W_B, Cm = x @ W_C                    (T, H, N=64)
  y  = selective scan over T (Mamba2 SSD)       (T, D)
  z  = relu(y @ W_yo + b_yo)                    (T, U=256)
  out = z @ W_head + b_head                     (T, A=64)

The scan uses the chunked (segsum) SSD formulation: chunk length L=128,
4 chunks, 16 independent heads. Per head/chunk:
  E[j,i]  = exp(Pcum_i - Pcum_j + log dt_j), causally masked to j<=i
  Y^T     = x_chunk^T Gw + S_prev^T (C*u),  Gw = (B C^T)^T . E, u_i=exp(Pcum_i)
  S_new   = exp(Pcum_L-1) S_prev + sum_j exp(Pcum_L-1 - Pcum_j) dt_j B_j x_j^T
Big matmuls run in bf16 with fp32 PSUM accumulation; the Diff matrix
(Pcum_i - Pcum_j + logdt_j) is built exactly with K=4 bf16 hi/lo-split
matmuls from operand packs bulk-staged once after the dt chain.

v4 scheduling (the performance-critical part):
  Warm matmuls pipeline at the theoretical array rate (no per-instruction
  overhead), but the PE clock halves (HAM K=4/8) whenever matmul density
  drops for a few microseconds.  So the whole kernel is arranged as one
  dense PE stream: scan head-quads are processed round-by-round with the
  NEXT quad's B/C projections (the dense GEMMs) interleaved between scan
  groups, and the z-projection interleaved into the final round.  The
  head-pair G matmuls are batched via the naturally-stacked projection
  psum layout (bm2 + zero-padded block-diag cm2z), which also removes the
  B-repack DMAs.  All scan operand staging is bulk (16 contiguous fills),
  since each DMA costs ~600ns of issuing-engine time.

Hardware notes (all discovered the hard way on this container's stack):
  - walrus here allows only ONE sync wait per instruction -> _split_multi_waits
  - compute-engine APs (matmul operands, DVE/ACT/gpsimd in or out) must
    start at partition 0 — anything else fails BIR verification or crashes
    the exec unit.  Only DMA may address other partitions, so B/C heads
    are repacked to base-0 tensors via bf16 staging + SBUF-to-SBUF DMA.
  - Softplus shares no ACT function table with Exp/Ln -> ln(1+exp(x)).
  - PSUM pools are bank-granular per tag: keep the baseline 8-bank layout.
"""

import sys
import types

import numpy as np
import ml_dtypes

T, BSZ, OBSD = 512, 8, 256
D, H, N, P = 2048, 16, 64, 128
U, A = 256, 64
L, NCH, KT = 128, 4, 16  # chunk length, #chunks, #d-tiles (D/128)
HN = H * N
BF16 = ml_dtypes.bfloat16

_CACHE = {}
_EXECUTED = {}


def _patch_tile():
    """Split the TileContext final drain's waits across single-wait nops."""
    from concourse import tile, mybir
    from concourse.vector_clock import ScopedClock

    if getattr(tile.TileContext, "_drain_patched", False):
        return

    def _patched(self, tick_clock, wait_clock):
        nc = self.nc
        probe = nc.sync.nop()
        wait_clock.add_sem_waits(
            probe.ins, ScopedClock({None: tick_clock.global_clock})
        )
        si = probe.ins.sync_info
        if si is not None and len(si.on_wait) > 1:
            waits = list(si.on_wait)
            probe.ins.sync_info = mybir.SyncInfo(
                on_wait=[waits[0]], on_update=list(si.on_update)
            )
            for w in waits[1:]:
                nop = nc.sync.nop()
                nop.ins.sync_info = mybir.SyncInfo(on_wait=[w], on_update=[])
        nc.sync.drain()
        nc.all_engine_barrier(sem_only=True)
        assert self.sems is not None
        popped = nc._tile_sem_poison_stack.pop()
        assert popped is self._sem_poison
        nc.clear_and_free_semaphores(list(self.sems.allocated().values()))
        nc.all_engine_barrier(sem_only=True)

    tile.TileContext._drain_and_barrier = _patched
    tile.TileContext._drain_patched = True


def _split_multi_waits(nc):
    """This walrus build accepts at most one sync wait per instruction.
    Hoist extra waits onto single-wait NoOps inserted just before, on the
    same engine (the sequencer stalls there first — strictly conservative)."""
    from concourse import mybir

    n = 0
    for f in nc.m.functions:
        for bb in f.blocks:
            insts = list(bb.instructions)
            changed = False
            new = []
            for inst in insts:
                try:
                    si = inst.sync_info
                except Exception:
                    si = None
                if si is not None and len(si.on_wait) > 1:
                    waits = list(si.on_wait)
                    for w in waits[:-1]:
                        nop = mybir.InstNoOp(
                            name=f"wsplit-{n}", ins=[], outs=[], engine=inst.engine
                        )
                        n += 1
                        nop.sync_info = mybir.SyncInfo(on_wait=[w], on_update=[])
                        nc.register_instruction(nop, overwrite=True)
                        new.append(nop)
                    inst.sync_info = mybir.SyncInfo(
                        on_wait=[waits[-1]], on_update=list(si.on_update)
                    )
                    changed = True
                new.append(inst)
            if changed:
                bb.instructions = new


def _inject_axon_hooks():
    """Make trace=True work (and a BASS_TRACE env var safe) in this container."""
    if "antenv.axon_hooks" not in sys.modules:
        try:
            from trn_agent_boot.trn_boot import _ntff_profile_via_ctypes

            hook = _ntff_profile_via_ctypes("/opt/axon/libaxon_pjrt.so")
        except Exception:
            hook = None
        mod = types.ModuleType("antenv.axon_hooks")
        mod.get_axon_ntff_profile_hook = lambda: hook
        mod.set_axon_ntff_profile_hook = lambda h: None
        sys.modules["antenv.axon_hooks"] = mod
    from concourse import bass_utils

    bass_utils.upload_artifacts = lambda d: d


def _build(with_b_in, with_b_yo, with_b_head):
    import concourse.bass as bass
    import concourse.mybir as mybir
    from concourse.tile import TileContext
    from concourse.masks import make_identity

    dt = mybir.dt
    AF = mybir.ActivationFunctionType
    OP = mybir.AluOpType

    nc = bass.Bass()
    obsT_e = nc.declare_dram_parameter("obsT", [OBSD, T], dt.bfloat16, isOutput=False)
    w_in_e = nc.declare_dram_parameter("w_in", [OBSD, D], dt.bfloat16, isOutput=False)
    w_dt_e = nc.declare_dram_parameter("w_dt", [D, H], dt.bfloat16, isOutput=False)
    w_b_e = nc.declare_dram_parameter("w_b", [D, HN], dt.bfloat16, isOutput=False)
    w_c_e = nc.declare_dram_parameter("w_c", [D, HN], dt.bfloat16, isOutput=False)
    w_yo_e = nc.declare_dram_parameter("w_yo", [D, U], dt.bfloat16, isOutput=False)
    w_hd_e = nc.declare_dram_parameter("w_hd", [U, A], dt.bfloat16, isOutput=False)
    neg_a_e = nc.declare_dram_parameter("neg_a", [H, 1], dt.float32, isOutput=False)
    dtb_e = nc.declare_dram_parameter("dtb", [H, 1], dt.float32, isOutput=False)
    bin_e = byo_e = bhd_e = None
    if with_b_in:
        bin_e = nc.declare_dram_parameter("b_in", [P, KT], dt.float32, isOutput=False)
    if with_b_yo:
        byo_e = nc.declare_dram_parameter("b_yo", [P, 2], dt.float32, isOutput=False)
    if with_b_head:
        bhd_e = nc.declare_dram_parameter("b_hd", [1, A], dt.bfloat16, isOutput=False)
    out_e = nc.declare_dram_parameter("out", [T, A], dt.float32, isOutput=True)

    _patch_tile()
    with TileContext(nc) as tc:
        with (
            tc.tile_pool(name="cst", bufs=1) as cst,
            tc.tile_pool(name="wrk", bufs=2) as wrk,
            tc.tile_pool(name="ps_proj", bufs=2, space="PSUM") as ps_proj,
            tc.tile_pool(name="ps_diff", bufs=2, space="PSUM") as ps_diff,
            tc.tile_pool(name="ps_gy", bufs=2, space="PSUM") as ps_gy,
            tc.tile_pool(name="ps_sd", bufs=1, space="PSUM") as ps_sd,
            tc.tile_pool(name="ps_tiny", bufs=1, space="PSUM") as ps_tiny,
        ):
            # ---------------- weights + constants -----------------
            # obsT/w_in halves split across both HWDGE queues so phase 1
            # starts as early as possible.
            obsT = cst.tile([P, 2 * T], dt.bfloat16, tag="obsT")
            nc.sync.dma_start(out=obsT[:, 0:T], in_=obsT_e[0:P, :])
            nc.scalar.dma_start(out=obsT[:, T : 2 * T], in_=obsT_e[P : 2 * P, :])
            w_in = cst.tile([P, 2 * D], dt.bfloat16, tag="w_in")
            nc.sync.dma_start(out=w_in[:, 0:D], in_=w_in_e[0:P, :])
            nc.scalar.dma_start(out=w_in[:, D : 2 * D], in_=w_in_e[P : 2 * P, :])
            w_dt = cst.tile([P, KT * H], dt.bfloat16, tag="w_dt")
            nc.sync.dma_start(
                out=w_dt[:].rearrange("p (k h) -> p k h", k=KT),
                in_=w_dt_e.rearrange("(k p) h -> p k h", p=P),
            )
            w_yo = cst.tile([P, KT * U], dt.bfloat16, tag="w_yo")
            nc.scalar.dma_start(
                out=w_yo[:].rearrange("p (k u) -> p k u", k=KT),
                in_=w_yo_e.rearrange("(k p) u -> p k u", p=P),
            )
            w_hd = cst.tile([P, 2 * A], dt.bfloat16, tag="w_hd")
            nc.scalar.dma_start(
                out=w_hd[:].rearrange("p (k a) -> p k a", k=2),
                in_=w_hd_e.rearrange("(k p) a -> p k a", p=P),
            )
            neg_a = cst.tile([H, 1], dt.float32, tag="neg_a")
            nc.sync.dma_start(out=neg_a[:], in_=neg_a_e[:])
            dtb = cst.tile([H, 1], dt.float32, tag="dtb")
            nc.sync.dma_start(out=dtb[:], in_=dtb_e[:])
            if with_b_in:
                b_in = cst.tile([P, KT], dt.float32, tag="b_in")
                nc.sync.dma_start(out=b_in[:], in_=bin_e[:])
            if with_b_yo:
                b_yo = cst.tile([P, 2], dt.float32, tag="b_yo")
                nc.sync.dma_start(out=b_yo[:], in_=byo_e[:])
            if with_b_head:
                b_hd = cst.tile([1, A], dt.bfloat16, tag="b_hd")
                nc.sync.dma_start(out=b_hd[:], in_=bhd_e[:])

            ident_f = cst.tile([H, H], dt.float32, tag="ident_f")
            make_identity(nc, ident_f[:])
            ident_pb = cst.tile([P, P], dt.bfloat16, tag="ident_pb")
            make_identity(nc, ident_pb[:])
            # causal ones: UT[j,i] = 1 where j<=i (cumsum matmul)
            ut_ones = cst.tile([L, L], dt.float32, tag="ut_ones")
            nc.gpsimd.memset(ut_ones[:], 1.0)
            nc.gpsimd.affine_select(
                out=ut_ones[:],
                in_=ut_ones[:],
                compare_op=OP.is_ge,
                fill=0.0,
                base=0,
                pattern=[[1, L]],
                channel_multiplier=-1,
            )
            ones_row = cst.tile([1, P], dt.float32, tag="ones_row")
            nc.gpsimd.memset(ones_row[:], 1.0)
            # head-pair row selector: ubp = sel2^T @ urow replicates u_he to
            # partitions 0..63 and u_ho to 64..127 in ONE matmul.  Built with
            # two affine_selects (memset can't start at partition 1).
            sel2 = cst.tile([2, P], dt.bfloat16, tag="sel2")
            nc.gpsimd.memset(sel2[:], 1.0)
            nc.gpsimd.affine_select(
                out=sel2[:], in_=sel2[:], compare_op=OP.is_ge, fill=0.0,
                base=0, pattern=[[1, P]], channel_multiplier=-N,
            )
            nc.gpsimd.affine_select(
                out=sel2[:], in_=sel2[:], compare_op=OP.is_ge, fill=0.0,
                base=N - 1, pattern=[[-1, P]], channel_multiplier=N,
            )

            # ---------------- activations / state ------------------
            xT = cst.tile([P, KT * T], dt.bfloat16, tag="xT")  # (d, t)
            x = cst.tile([P, NCH * D], dt.bfloat16, tag="x")  # (t, d) per t-tile
            y = cst.tile([P, KT * T], dt.bfloat16, tag="y")  # (d, t) per d-tile
            zT = obsT  # reuse: obs fully consumed by phase 1
            logit = cst.tile([P, NCH * A], dt.float32, tag="logit")
            # B stacked per head-pair: pair mt at cols [mt*T,(mt+1)*T), rows
            # 0..63 = head 2mt (n), 64..127 = head 2mt+1.  This is exactly the
            # projection psum layout, so evac is a single DVE copy (no DMA).
            bm2 = cst.tile([P, 8 * T], dt.bfloat16, tag="bm2")
            # odd heads' (n,t) at base partition 0 (2-deep manual ring by mt)
            bmho = cst.tile([N, 2 * T], dt.bfloat16, tag="bmho")
            # Cw per head (n, t): head h at cols [h*T,(h+1)*T)
            cw = cst.tile([N, H * T], dt.bfloat16, tag="cw")
            # zero-padded block-diag C for the pair-batched G matmuls.  Manual
            # 2-deep ring by quad: half q%2 at cols [half*2048,(half+1)*2048),
            # layout (pair-in-quad, chunk, head-in-pair, t); zeros persist.
            cm2z = cst.tile([P, 2 * 2048], dt.bfloat16, tag="cm2z")
            nc.gpsimd.memset(cm2z[:], 0.0)
            # B^T (t, n) blocks for the S update.  Ring half per quad,
            # layout (hi, chunk, n): col = half*1024 + hi*256 + c*64 + n.
            bmt2 = cst.tile([P, 2 * 1024], dt.bfloat16, tag="bmt2")
            s_st = [
                cst.tile([N, P], dt.bfloat16, tag=f"s{h}", name=f"s{h}")
                for h in range(H)
            ]
            for h in range(H):
                nc.gpsimd.memset(s_st[h][:], 0.0)

            dtT = cst.tile([H, T], dt.float32, tag="dtT")
            logdtT = cst.tile([H, T], dt.float32, tag="logdtT")
            pcumT = cst.tile([H, T], dt.float32, tag="pcumT")  # Pcum rows (h,t)
            # two-float (hi/lo bf16) splits feeding the K=4 Diff matmuls
            pcumH = cst.tile([H, T], dt.bfloat16, tag="pcumH")
            pcumL = cst.tile([H, T], dt.bfloat16, tag="pcumL")
            npdH = cst.tile([H, T], dt.bfloat16, tag="npdH")
            npdL = cst.tile([H, T], dt.bfloat16, tag="npdL")
            plrow = cst.tile([1, NCH * H], dt.float32, tag="plrow")
            u_all = cst.tile([H, T], dt.bfloat16, tag="u_all")  # exp(Pcum)
            cols = cst.tile([P, NCH * 2 * H], dt.float32, tag="cols")  # [PcumCol|dtCol]
            e2c = cst.tile([P, NCH * H], dt.float32, tag="e2c")  # exp(Plast-Pcum)
            dtotc = cst.tile([P, NCH * H], dt.float32, tag="dtotc")
            ulast = cst.tile([1, NCH * H], dt.float32, tag="ulast")
            # K=4 Diff operand packs for ALL (chunk, head) tiles: cols
            # (c, h, t); group (c,hg) slices a contiguous (4, 4L) window.
            # lh rows [1, npdH, 1, npdL]; rp rows [pcumH, 1, pcumL, 1].
            lh_all = cst.tile([4, NCH * H * L], dt.bfloat16, tag="lh_all")
            rp_all = cst.tile([4, NCH * H * L], dt.bfloat16, tag="rp_all")
            nc.gpsimd.memset(lh_all[:], 1.0)
            nc.gpsimd.memset(rp_all[:], 1.0)

            # ---------------- x^T = relu(W_in^T obs^T) (d,t) --------
            for kt in range(KT):
                ps = ps_proj.tile([P, T], dt.float32, tag="proj")
                for ko in range(2):
                    nc.tensor.matmul(
                        ps[:],
                        w_in[:, ko * D + kt * P : ko * D + (kt + 1) * P],
                        obsT[:, ko * T : (ko + 1) * T],
                        start=(ko == 0),
                        stop=(ko == 1),
                    )
                if with_b_in:
                    nc.scalar.activation(
                        xT[:, kt * T : (kt + 1) * T], ps[:], AF.Relu,
                        bias=b_in[:, kt : kt + 1],
                    )
                else:
                    nc.scalar.activation(xT[:, kt * T : (kt + 1) * T], ps[:], AF.Relu)

            # ---------------- x = xT^T via PE transposes (t,d) ------
            for tt in range(NCH):
                for g4 in range(4):
                    ps = ps_proj.tile([P, 4 * P], dt.bfloat16, tag="proj")
                    for kk in range(4):
                        kt = g4 * 4 + kk
                        nc.tensor.transpose(
                            ps[:, kk * P : (kk + 1) * P],
                            xT[:, kt * T + tt * P : kt * T + (tt + 1) * P],
                            ident_pb[:],
                        )
                    nc.vector.tensor_copy(
                        x[:, tt * D + g4 * 512 : tt * D + (g4 + 1) * 512], ps[:]
                    )

            # ---------------- dt chain ------------------------------
            psd = ps_proj.tile([H, T], dt.float32, tag="proj", name="psd")
            for kt in range(KT):
                nc.tensor.matmul(
                    psd[:],
                    w_dt[:, kt * H : (kt + 1) * H],
                    xT[:, kt * T : (kt + 1) * T],
                    start=(kt == 0),
                    stop=(kt == KT - 1),
                )
            # softplus via ln(1+exp(.)) — Softplus shares no ACT table with
            # Exp/Ln on this compiler; exp/ln/relu/copy live in one table.
            ez = ps_diff.tile([H, T], dt.float32, tag="diff", name="ez")
            nc.scalar.activation(ez[:], psd[:], AF.Exp, bias=dtb[:])
            nc.vector.tensor_scalar_add(ez[:], ez[:], 1.0)
            nc.scalar.activation(dtT[:], ez[:], AF.Ln)
            nc.scalar.activation(logdtT[:], dtT[:], AF.Ln)

            def prep_chunk(c):
                cb = slice(c * L, (c + 1) * L)
                ldec = wrk.tile([H, L], dt.float32, tag="ldec")
                nc.vector.tensor_scalar_mul(ldec[:], dtT[:, cb], neg_a[:])
                pt = ps_tiny.tile([P, 2 * H], dt.float32, tag="tiny")
                nc.tensor.transpose(pt[:, 0:H], ldec[:], ident_f[:])
                ldec_c = wrk.tile([P, H], dt.float32, tag="ldec_c")
                nc.vector.tensor_copy(ldec_c[:], pt[:, 0:H])
                pp = ps_tiny.tile([H, L], dt.float32, tag="tiny")
                nc.tensor.matmul(pp[:], ldec_c[:], ut_ones[:], start=True, stop=True)
                nc.vector.tensor_copy(pcumT[:, cb], pp[:])
                npdc = wrk.tile([H, L], dt.float32, tag="npdc")
                nc.vector.tensor_sub(npdc[:], logdtT[:, cb], pcumT[:, cb])
                nc.vector.tensor_copy(pcumH[:, cb], pcumT[:, cb])
                nc.vector.tensor_sub(pcumL[:, cb], pcumT[:, cb], pcumH[:, cb])
                nc.vector.tensor_copy(npdH[:, cb], npdc[:])
                nc.vector.tensor_sub(npdL[:, cb], npdc[:], npdH[:, cb])
                nc.scalar.activation(u_all[:, cb], pcumT[:, cb], AF.Exp)
                pt2 = ps_tiny.tile([P, 2 * H], dt.float32, tag="tiny")
                nc.tensor.transpose(pt2[:, 0:H], pcumT[:, cb], ident_f[:])
                nc.tensor.transpose(pt2[:, H : 2 * H], dtT[:, cb], ident_f[:])
                co = c * 2 * H
                nc.vector.tensor_copy(cols[:, co : co + 2 * H], pt2[:])
                # PcumLast per head at base partition 0 (row 127 of PcumCol)
                nc.sync.dma_start(
                    out=plrow[:, c * H : (c + 1) * H],
                    in_=cols[L - 1 : L, co : co + H],
                )
                plast = plrow[:, c * H : (c + 1) * H]
                nc.scalar.activation(ulast[:, c * H : (c + 1) * H], plast, AF.Exp)
                pdt = ps_tiny.tile([P, H], dt.float32, tag="tiny")
                nc.tensor.matmul(
                    pdt[:], ones_row[0:1, 0:P], ulast[:, c * H : (c + 1) * H],
                    start=True, stop=True,
                )
                nc.vector.tensor_copy(dtotc[:, c * H : (c + 1) * H], pdt[:])
                ppl = ps_tiny.tile([P, H], dt.float32, tag="tiny")
                nc.tensor.matmul(
                    ppl[:], ones_row[0:1, 0:P], plast, start=True, stop=True
                )
                e2a = wrk.tile([P, H], dt.float32, tag="e2a")
                nc.vector.tensor_sub(e2a[:], ppl[:], cols[:, co : co + H])
                nc.scalar.activation(e2c[:, c * H : (c + 1) * H], e2a[:], AF.Exp)

            # ---------------- B / C projections ---------------------
            # pair j covers mts (2j, 2j+1) = head quad j; one contiguous
            # 256-col weight load per pair.
            wbv = w_b_e.rearrange("(kt p) (j m) -> p kt j m", p=P, j=4)
            wcv = w_c_e.rearrange("(kt p) (j m) -> p kt j m", p=P, j=4)

            def emit_wload(j):
                wbuf2 = wrk.tile([P, KT * 2 * P], dt.bfloat16, tag="wbs", bufs=2)
                nc.sync.dma_start(
                    out=wbuf2[:].rearrange("p (kt m) -> p kt m", kt=KT),
                    in_=wbv[:, :, j, :],
                )
                wcuf2 = wrk.tile([P, KT * 2 * P], dt.bfloat16, tag="wcs", bufs=2)
                nc.scalar.dma_start(
                    out=wcuf2[:].rearrange("p (kt m) -> p kt m", kt=KT),
                    in_=wcv[:, :, j, :],
                )
                return wbuf2, wcuf2

            def emit_projBC(mt, wbuf2, wcuf2):
                u = mt % 2
                half = (mt // 2) % 2
                psb = ps_proj.tile([P, T], dt.float32, tag="proj")
                for kt in range(KT):
                    nc.tensor.matmul(
                        psb[:],
                        wbuf2[:, kt * 2 * P + u * P : kt * 2 * P + (u + 1) * P],
                        xT[:, kt * T : (kt + 1) * T],
                        start=(kt == 0),
                        stop=(kt == KT - 1),
                    )
                nc.vector.tensor_copy(bm2[:, mt * T : (mt + 1) * T], psb[:])
                nc.sync.dma_start(
                    out=bmho[:, u * T : (u + 1) * T],
                    in_=bm2[N:P, mt * T : (mt + 1) * T],
                )
                psc = ps_proj.tile([P, T], dt.float32, tag="proj")
                for kt in range(KT):
                    nc.tensor.matmul(
                        psc[:],
                        wcuf2[:, kt * 2 * P + u * P : kt * 2 * P + (u + 1) * P],
                        xT[:, kt * T : (kt + 1) * T],
                        start=(kt == 0),
                        stop=(kt == KT - 1),
                    )
                ctmp = wrk.tile([P, T], dt.bfloat16, tag="ctmp", bufs=2)
                nc.vector.tensor_copy(ctmp[:], psc[:])
                # block-diag C fills (zeros persist in the off-diag blocks)
                vh = cm2z[0:N, half * 2048 : (half + 1) * 2048].rearrange(
                    "n (pp c q t) -> n pp c q t", pp=2, c=NCH, q=2
                )
                nc.sync.dma_start(
                    out=vh[:, u : u + 1, :, 0:1, :],
                    in_=ctmp[0:N, :].rearrange("n (c t) -> n c t", c=NCH),
                )
                vl = cm2z[N:P, half * 2048 : (half + 1) * 2048].rearrange(
                    "n (pp c q t) -> n pp c q t", pp=2, c=NCH, q=2
                )
                nc.scalar.dma_start(
                    out=vl[:, u : u + 1, :, 1:2, :],
                    in_=ctmp[N:P, :].rearrange("n (c t) -> n c t", c=NCH),
                )
                # B^T (t,n) blocks: head 2mt from bm2 rows 0..63, head 2mt+1
                # from the base-0 bmho copy.
                ptr = ps_tiny.tile([P, 4 * P], dt.bfloat16, tag="tiny")
                for q in range(2):
                    for c in range(NCH):
                        if q == 0:
                            src = bm2[0:N, mt * T + c * L : mt * T + (c + 1) * L]
                        else:
                            src = bmho[:, u * T + c * L : u * T + (c + 1) * L]
                        nc.tensor.transpose(
                            ptr[:, q * NCH * N + c * N : q * NCH * N + (c + 1) * N],
                            src,
                            ident_pb[0:N, 0:N],
                        )
                nc.vector.tensor_copy(
                    bmt2[:, half * 1024 + u * 512 : half * 1024 + (u + 1) * 512],
                    ptr[:],
                )
                return ctmp

            def emit_cw(mt, ctmp):
                he, ho = 2 * mt, 2 * mt + 1
                urow = wrk.tile([2, T], dt.bfloat16, tag="urow")
                nc.scalar.dma_start(out=urow[:], in_=u_all[he : ho + 1, :])
                ubp = ps_diff.tile([P, 4 * L], dt.float32, tag="diff", name="ubp")
                nc.tensor.matmul(ubp[:], sel2[:], urow[:], start=True, stop=True)
                ubc = wrk.tile([P, T], dt.bfloat16, tag="ubc", bufs=2)
                nc.scalar.activation(ubc[:], ubp[:], AF.Copy)
                wtmp = wrk.tile([P, T], dt.bfloat16, tag="wtmp", bufs=2)
                nc.vector.tensor_mul(wtmp[:], ctmp[:], ubc[:])
                nc.sync.dma_start(out=cw[:, he * T : (he + 1) * T], in_=wtmp[0:N, :])
                nc.scalar.dma_start(out=cw[:, ho * T : (ho + 1) * T], in_=wtmp[N:P, :])

            # prep interleaved with the first pair's projections
            wl = {0: emit_wload(0)}
            prep_chunk(0)
            prep_chunk(1)
            ct0 = emit_projBC(0, *wl[0])
            prep_chunk(2)
            prep_chunk(3)
            ct1 = emit_projBC(1, *wl[0])
            wl[1] = emit_wload(1)
            # bulk-stage the Diff packs: lh rows 1/3 <- npd hi/lo, rp rows
            # 0/2 <- pcum hi/lo; one contiguous (16,L) fill per (row, chunk).
            for c in range(NCH):
                cb = slice(c * L, (c + 1) * L)
                for row, src, eng in (
                    (1, npdH, nc.sync),
                    (3, npdL, nc.scalar),
                ):
                    eng.dma_start(
                        out=lh_all[row : row + 1, c * H * L : (c + 1) * H * L]
                        .rearrange("p (h t) -> p h t", h=H),
                        in_=src[:, cb],
                    )
                for row, src, eng in (
                    (0, pcumH, nc.sync),
                    (2, pcumL, nc.scalar),
                ):
                    eng.dma_start(
                        out=rp_all[row : row + 1, c * H * L : (c + 1) * H * L]
                        .rearrange("p (h t) -> p h t", h=H),
                        in_=src[:, cb],
                    )
            emit_cw(0, ct0)
            emit_cw(1, ct1)

            # ---------------- scan ----------------------------------
            yv = y[:].rearrange("p (h t) -> p h t", h=KT)
            # e_sb / gw rings carved from the dead w_in tile (bf16,
            # (P, 4096) = 8 x 512-col slots).  w_in's last read is phase 1.
            esb_ring = [w_in[:, i * 512 : (i + 1) * 512] for i in range(3)]
            gw_ring = [w_in[:, (3 + i) * 512 : (4 + i) * 512] for i in range(3)]
            st = {"pend": None, "g": 0, "z": 0}

            def z_head(h):
                for ut in range(2):
                    nc.tensor.matmul(
                        z_ps[ut][:],
                        w_yo[:, h * U + ut * P : h * U + (ut + 1) * P],
                        y[:, h * T : (h + 1) * T],
                        start=(st["z"] == 0),
                        stop=(st["z"] == H - 1),
                    )
                st["z"] += 1

            def group_front(c, hg):
                half = hg % 2
                dbank = ps_diff.tile([P, 4 * L], dt.float32, tag="diff")
                for hi in range(4):
                    h = 4 * hg + hi
                    sl = slice(c * H * L + h * L, c * H * L + (h + 1) * L)
                    nc.tensor.matmul(
                        dbank[:, hi * L : (hi + 1) * L],
                        lh_all[:, sl],
                        rp_all[:, sl],
                        start=True,
                        stop=True,
                    )
                gbank = ps_gy.tile([P, 4 * L], dt.float32, tag="gy")
                for pq in range(2):
                    mt = 2 * hg + pq
                    nc.tensor.matmul(
                        gbank[:, pq * 2 * L : (pq + 1) * 2 * L],
                        bm2[:, mt * T + c * L : mt * T + (c + 1) * L],
                        cm2z[
                            :,
                            half * 2048 + pq * 1024 + c * 256 : half * 2048
                            + pq * 1024
                            + (c + 1) * 256,
                        ],
                        start=True,
                        stop=True,
                    )
                gi = st["g"]
                e_sb = esb_ring[gi % 3]
                nc.scalar.activation(e_sb, dbank[:], AF.Exp)
                # causal mask: keep i>=j else 0 (kills the exp-overflow infs)
                nc.gpsimd.affine_select(
                    out=e_sb,
                    in_=e_sb,
                    compare_op=OP.is_ge,
                    fill=0.0,
                    base=0,
                    pattern=[[0, 4], [1, L]],
                    channel_multiplier=-1,
                )
                gw = gw_ring[gi % 3]
                nc.vector.tensor_mul(gw, gbank[:], e_sb)
                if st["pend"] is not None:
                    st["pend"]()

                def consume(c=c, hg=hg, gw=gw, half=half):
                    ybank = ps_gy.tile([P, 4 * L], dt.float32, tag="gy")
                    sdb = ps_sd.tile([N, 4 * P], dt.float32, tag="sd")
                    bd = wrk.tile([P, 4 * N], dt.bfloat16, tag="bd")
                    for hi in range(4):
                        h = 4 * hg + hi
                        xc = x[:, c * D + h * P : c * D + (h + 1) * P]
                        nc.tensor.matmul(
                            ybank[:, hi * L : (hi + 1) * L],
                            xc,
                            gw[:, hi * L : (hi + 1) * L],
                            start=True,
                            stop=False,
                        )
                        nc.tensor.matmul(
                            ybank[:, hi * L : (hi + 1) * L],
                            s_st[h][:],
                            cw[:, h * T + c * L : h * T + (c + 1) * L],
                            start=False,
                            stop=True,
                        )
                        nc.vector.tensor_scalar(
                            bd[:, hi * N : (hi + 1) * N],
                            bmt2[
                                :,
                                half * 1024 + hi * 256 + c * N : half * 1024
                                + hi * 256
                                + (c + 1) * N,
                            ],
                            e2c[:, c * H + h : c * H + h + 1],
                            cols[:, c * 2 * H + H + h : c * 2 * H + H + h + 1],
                            op0=OP.mult,
                            op1=OP.mult,
                        )
                        sds = sdb[:, hi * P : (hi + 1) * P]
                        nc.tensor.matmul(
                            sds, bd[:, hi * N : (hi + 1) * N], xc,
                            start=True, stop=True,
                        )
                        nc.vector.scalar_tensor_tensor(
                            s_st[h][:],
                            s_st[h][:],
                            dtotc[0:N, c * H + h : c * H + h + 1],
                            sds,
                            op0=OP.mult,
                            op1=OP.add,
                        )
                    # Y evac: psum (p, (hi,L)) -> y cols (4*hg+hi, c*L..)
                    nc.scalar.activation(
                        yv[:, 4 * hg : 4 * hg + 4, c * L : (c + 1) * L],
                        ybank[:].rearrange("p (i t) -> p i t", i=4),
                        AF.Copy,
                    )

                st["pend"] = consume
                st["g"] += 1

            # z emission order for the final round: quad-2 heads must come
            # after consume(3,2), which fires at group (0,3)'s front.
            zq3 = [0, 1, 2, 3, 4, 5, 6, 7, 8, 9, 10, 11]
            for hg in range(4):
                for c in range(NCH):
                    group_front(c, hg)
                    if hg < 3:
                        if c == 1:
                            ct_a = emit_projBC(2 * hg + 2, *wl[hg + 1])
                        elif c == 2:
                            ct_b = emit_projBC(2 * hg + 3, *wl[hg + 1])
                        elif c == 3:
                            emit_cw(2 * hg + 2, ct_a)
                            emit_cw(2 * hg + 3, ct_b)
                            if hg + 2 <= 3:
                                wl[hg + 2] = emit_wload(hg + 2)
                    else:
                        for k in range(3):
                            z_head(zq3[c * 3 + k])
                if hg == 2:
                    z_ps = [
                        ps_proj.tile([P, T], dt.float32, tag="proj", name=f"z{ut}")
                        for ut in range(2)
                    ]
            st["pend"]()
            for h in (12, 13, 14, 15):
                z_head(h)

            # ---------------- z = relu(y W_yo) (u,t) ----------------
            for ut in range(2):
                if with_b_yo:
                    nc.scalar.activation(
                        zT[:, ut * T : (ut + 1) * T], z_ps[ut][:], AF.Relu,
                        bias=b_yo[:, ut : ut + 1],
                    )
                else:
                    nc.scalar.activation(
                        zT[:, ut * T : (ut + 1) * T], z_ps[ut][:], AF.Relu
                    )

            # ---------------- logits --------------------------------
            for tt in range(NCH):
                ps = ps_proj.tile([P, A], dt.float32, tag="proj")
                nmm = 3 if with_b_head else 2
                for ut in range(2):
                    nc.tensor.matmul(
                        ps[:],
                        zT[:, ut * T + tt * P : ut * T + (tt + 1) * P],
                        w_hd[:, ut * A : (ut + 1) * A],
                        start=(ut == 0),
                        stop=(ut == nmm - 1),
                    )
                if with_b_head:
                    nc.tensor.matmul(
                        ps[:],
                        lh_all[0:1, tt * P : (tt + 1) * P],
                        b_hd[:],
                        start=False,
                        stop=True,
                    )
                nc.scalar.activation(logit[:, tt * A : (tt + 1) * A], ps[:], AF.Copy)
                nc.sync.dma_start(
                    out=out_e[tt * P : (tt + 1) * P, :],
                    in_=logit[:, tt * A : (tt + 1) * A],
                )

    _split_multi_waits(nc)
    return nc


def kernel(obs, W_in, b_in, A_log, dt_bias, W_dt, W_B, W_C, W_yo, b_yo, W_head, b_head):
    _inject_axon_hooks()
    _patch_tile()
    from concourse.bass_utils import run_bass_kernel_spmd

    obs = np.asarray(obs, dtype=np.float32)
    flags = (
        bool(np.any(np.asarray(b_in) != 0)),
        bool(np.any(np.asarray(b_yo) != 0)),
        bool(np.any(np.asarray(b_head) != 0)),
    )
    # First call: build once (the verified path). Repeat calls in one
    # process rebuild a fresh graph — re-executing a previously-run nc with
    # new inputs has crashed the exec unit (NRT status 101) in testing.
    if flags not in _CACHE:
        _CACHE[flags] = _build(*flags)
    elif _EXECUTED.get(flags):
        _CACHE[flags] = _build(*flags)
    nc = _CACHE[flags]
    _EXECUTED[flags] = True

    obsT = obs.reshape(T, BSZ, OBSD).transpose(1, 2, 0)  # (B, 256, T)
    base = {
        "w_in": np.ascontiguousarray(W_in).astype(BF16),
        "w_dt": np.ascontiguousarray(W_dt).astype(BF16),
        "w_b": np.ascontiguousarray(W_B).astype(BF16),
        "w_c": np.ascontiguousarray(W_C).astype(BF16),
        "w_yo": np.ascontiguousarray(W_yo).astype(BF16),
        "w_hd": np.ascontiguousarray(W_head).astype(BF16),
        "neg_a": (-np.exp(np.asarray(A_log, np.float64)))
        .astype(np.float32)
        .reshape(H, 1),
        "dtb": np.asarray(dt_bias, np.float32).reshape(H, 1),
    }
    if flags[0]:
        base["b_in"] = np.ascontiguousarray(
            np.asarray(b_in, np.float32).reshape(KT, P).T
        )
    if flags[1]:
        base["b_yo"] = np.ascontiguousarray(
            np.asarray(b_yo, np.float32).reshape(2, P).T
        )
    if flags[2]:
        base["b_hd"] = np.asarray(b_head).astype(BF16).reshape(1, A)
    in_maps = [
        dict(base, obsT=np.ascontiguousarray(obsT[c]).astype(BF16)) for c in range(BSZ)
    ]
    global _last_in_maps
    _last_in_maps = in_maps
    res = run_bass_kernel_spmd(nc, in_maps, core_ids=list(range(BSZ)))
    out = np.stack([res.results[c]["out"] for c in range(BSZ)], axis=1)
    return out.astype(np.float32)


# revision 30
# speedup vs baseline: 1.1575x; 1.0030x over previous
"""Self-contained Trainium2 kernel for the SSD-scan actor network.

Data-parallel over batch B=8 across 8 NeuronCores (one sample per core, no
collectives). Per core:
  x  = relu(obs @ W_in + b_in)                  (T=512, D=2048)
  dt = softplus(x @ W_dt + dt_bias)             (T, H=16)
  Bm = x @ W_B, Cm = x @ W_C                    (T, H, N=64)
  y  = selective scan over T (Mamba2 SSD)       (T, D)
  z  = relu(y @ W_yo + b_yo)                    (T, U=256)
  out = z @ W_head + b_head                     (T, A=64)

The scan uses the chunked (segsum) SSD formulation: chunk length L=128,
4 chunks, 16 independent heads. Per head/chunk:
  E[j,i]  = exp(Pcum_i - Pcum_j + log dt_j), causally masked to j<=i
  Y^T     = x_chunk^T Gw + S_prev^T (C*u),  Gw = (B C^T)^T . E, u_i=exp(Pcum_i)
  S_new   = exp(Pcum_L-1) S_prev + sum_j exp(Pcum_L-1 - Pcum_j) dt_j B_j x_j^T
Big matmuls run in bf16 with fp32 PSUM accumulation; the Diff matrix
(Pcum_i - Pcum_j + logdt_j) is built exactly with K=4 bf16 hi/lo-split
matmuls from operand packs bulk-staged once after the dt chain.

v4 scheduling (the performance-critical part):
  Warm matmuls pipeline at the theoretical array rate (no per-instruction
  overhead), but the PE clock halves (HAM K=4/8) whenever matmul density
  drops for a few microseconds.  So the whole kernel is arranged as one
  dense PE stream: scan head-quads are processed round-by-round with the
  NEXT quad's B/C projections (the dense GEMMs) interleaved between scan
  groups, and the z-projection interleaved into the final round.  The
  head-pair G matmuls are batched via the naturally-stacked projection
  psum layout (bm2 + zero-padded block-diag cm2z), which also removes the
  B-repack DMAs.  All scan operand staging is bulk (16 contiguous fills),
  since each DMA costs ~600ns of issuing-engine time.

Hardware notes (all discovered the hard way on this container's stack):
  - walrus here allows only ONE sync wait per instruction -> _split_multi_waits
  - compute-engine APs (matmul operands, DVE/ACT/gpsimd in or out) must
    start at partition 0 — anything else fails BIR verification or crashes
    the exec unit.  Only DMA may address other partitions, so B/C heads
    are repacked to base-0 tensors via bf16 staging + SBUF-to-SBUF DMA.
  - Softplus shares no ACT function table with Exp/Ln -> ln(1+exp(x)).
  - PSUM pools are bank-granular per tag: keep the baseline 8-bank layout.
"""

import sys
import types

import numpy as np
import ml_dtypes

T, BSZ, OBSD = 512, 8, 256
D, H, N, P = 2048, 16, 64, 128
U, A = 256, 64
L, NCH, KT = 128, 4, 16  # chunk length, #chunks, #d-tiles (D/128)
HN = H * N
BF16 = ml_dtypes.bfloat16

_CACHE = {}
_EXECUTED = {}


def _patch_tile():
    """Split the TileContext final drain's waits across single-wait nops."""
    from concourse import tile, mybir
    from concourse.vector_clock import ScopedClock

    if getattr(tile.TileContext, "_drain_patched", False):
        return

    def _patched(self, tick_clock, wait_clock):
        nc = self.nc
        probe = nc.sync.nop()
        wait_clock.add_sem_waits(
            probe.ins, ScopedClock({None: tick_clock.global_clock})
        )
        si = probe.ins.sync_info
        if si is not None and len(si.on_wait) > 1:
            waits = list(si.on_wait)
            probe.ins.sync_info = mybir.SyncInfo(
                on_wait=[waits[0]], on_update=list(si.on_update)
            )
            for w in waits[1:]:
                nop = nc.sync.nop()
                nop.ins.sync_info = mybir.SyncInfo(on_wait=[w], on_update=[])
        nc.sync.drain()
        nc.all_engine_barrier(sem_only=True)
        assert self.sems is not None
        popped = nc._tile_sem_poison_stack.pop()
        assert popped is self._sem_poison
        nc.clear_and_free_semaphores(list(self.sems.allocated().values()))
        nc.all_engine_barrier(sem_only=True)

    tile.TileContext._drain_and_barrier = _patched
    tile.TileContext._drain_patched = True


def _split_multi_waits(nc):
    """This walrus build accepts at most one sync wait per instruction.
    Hoist extra waits onto single-wait NoOps inserted just before, on the
    same engine (the sequencer stalls there first — strictly conservative)."""
    from concourse import mybir

    n = 0
    for f in nc.m.functions:
        for bb in f.blocks:
            insts = list(bb.instructions)
            changed = False
            new = []
            for inst in insts:
                try:
                    si = inst.sync_info
                except Exception:
                    si = None
                if si is not None and len(si.on_wait) > 1:
                    waits = list(si.on_wait)
                    for w in waits[:-1]:
                        nop = mybir.InstNoOp(
                            name=f"wsplit-{n}", ins=[], outs=[], engine=inst.engine
                        )
                        n += 1
                        nop.sync_info = mybir.SyncInfo(on_wait=[w], on_update=[])
                        nc.register_instruction(nop, overwrite=True)
                        new.append(nop)
                    inst.sync_info = mybir.SyncInfo(
                        on_wait=[waits[-1]], on_update=list(si.on_update)
                    )
                    changed = True
                new.append(inst)
            if changed:
                bb.instructions = new


def _inject_axon_hooks():
    """Make trace=True work (and a BASS_TRACE env var safe) in this container."""
    if "antenv.axon_hooks" not in sys.modules:
        try:
            from trn_agent_boot.trn_boot import _ntff_profile_via_ctypes

            hook = _ntff_profile_via_ctypes("/opt/axon/libaxon_pjrt.so")
        except Exception:
            hook = None
        mod = types.ModuleType("antenv.axon_hooks")
        mod.get_axon_ntff_profile_hook = lambda: hook
        mod.set_axon_ntff_profile_hook = lambda h: None
        sys.modules["antenv.axon_hooks"] = mod
    from concourse import bass_utils

    bass_utils.upload_artifacts = lambda d: d


def _build(with_b_in, with_b_yo, with_b_head):
    import concourse.bass as bass
    import concourse.mybir as mybir
    from concourse.tile import TileContext
    from concourse.masks import make_identity

    dt = mybir.dt
    AF = mybir.ActivationFunctionType
    OP = mybir.AluOpType

    nc = bass.Bass()
    obsT_e = nc.declare_dram_parameter("obsT", [OBSD, T], dt.bfloat16, isOutput=False)
    w_in_e = nc.declare_dram_parameter("w_in", [OBSD, D], dt.bfloat16, isOutput=False)
    w_dt_e = nc.declare_dram_parameter("w_dt", [D, H], dt.bfloat16, isOutput=False)
    w_b_e = nc.declare_dram_parameter("w_b", [D, HN], dt.bfloat16, isOutput=False)
    w_c_e = nc.declare_dram_parameter("w_c", [D, HN], dt.bfloat16, isOutput=False)
    w_yo_e = nc.declare_dram_parameter("w_yo", [D, U], dt.bfloat16, isOutput=False)
    w_hd_e = nc.declare_dram_parameter("w_hd", [U, A], dt.bfloat16, isOutput=False)
    neg_a_e = nc.declare_dram_parameter("neg_a", [H, 1], dt.float32, isOutput=False)
    dtb_e = nc.declare_dram_parameter("dtb", [H, 1], dt.float32, isOutput=False)
    bin_e = byo_e = bhd_e = None
    if with_b_in:
        bin_e = nc.declare_dram_parameter("b_in", [P, KT], dt.float32, isOutput=False)
    if with_b_yo:
        byo_e = nc.declare_dram_parameter("b_yo", [P, 2], dt.float32, isOutput=False)
    if with_b_head:
        bhd_e = nc.declare_dram_parameter("b_hd", [1, A], dt.bfloat16, isOutput=False)
    out_e = nc.declare_dram_parameter("out", [T, A], dt.float32, isOutput=True)

    _patch_tile()
    with TileContext(nc) as tc:
        with (
            tc.tile_pool(name="cst", bufs=1) as cst,
            tc.tile_pool(name="wrk", bufs=2) as wrk,
            tc.tile_pool(name="ps_proj", bufs=2, space="PSUM") as ps_proj,
            tc.tile_pool(name="ps_diff", bufs=2, space="PSUM") as ps_diff,
            tc.tile_pool(name="ps_gy", bufs=2, space="PSUM") as ps_gy,
            tc.tile_pool(name="ps_sd", bufs=1, space="PSUM") as ps_sd,
            tc.tile_pool(name="ps_tiny", bufs=1, space="PSUM") as ps_tiny,
        ):
            # ---------------- weights + constants -----------------
            # obsT/w_in halves split across both HWDGE queues so phase 1
            # starts as early as possible.
            obsT = cst.tile([P, 2 * T], dt.bfloat16, tag="obsT")
            nc.sync.dma_start(out=obsT[:, 0:T], in_=obsT_e[0:P, :])
            nc.scalar.dma_start(out=obsT[:, T : 2 * T], in_=obsT_e[P : 2 * P, :])
            w_in = cst.tile([P, 2 * D], dt.bfloat16, tag="w_in")
            nc.sync.dma_start(out=w_in[:, 0 : D // 2], in_=w_in_e[0:P, 0 : D // 2])
            nc.sync.dma_start(out=w_in[:, D // 2 : D], in_=w_in_e[0:P, D // 2 : D])
            nc.scalar.dma_start(
                out=w_in[:, D : D + D // 2], in_=w_in_e[P : 2 * P, 0 : D // 2]
            )
            nc.scalar.dma_start(
                out=w_in[:, D + D // 2 : 2 * D], in_=w_in_e[P : 2 * P, D // 2 : D]
            )
            w_dt = cst.tile([P, KT * H], dt.bfloat16, tag="w_dt")
            nc.sync.dma_start(
                out=w_dt[:].rearrange("p (k h) -> p k h", k=KT),
                in_=w_dt_e.rearrange("(k p) h -> p k h", p=P),
            )
            w_yo = cst.tile([P, KT * U], dt.bfloat16, tag="w_yo")
            nc.scalar.dma_start(
                out=w_yo[:].rearrange("p (k u) -> p k u", k=KT),
                in_=w_yo_e.rearrange("(k p) u -> p k u", p=P),
            )
            w_hd = cst.tile([P, 2 * A], dt.bfloat16, tag="w_hd")
            nc.scalar.dma_start(
                out=w_hd[:].rearrange("p (k a) -> p k a", k=2),
                in_=w_hd_e.rearrange("(k p) a -> p k a", p=P),
            )
            neg_a = cst.tile([H, 1], dt.float32, tag="neg_a")
            nc.sync.dma_start(out=neg_a[:], in_=neg_a_e[:])
            dtb = cst.tile([H, 1], dt.float32, tag="dtb")
            nc.sync.dma_start(out=dtb[:], in_=dtb_e[:])
            if with_b_in:
                b_in = cst.tile([P, KT], dt.float32, tag="b_in")
                nc.sync.dma_start(out=b_in[:], in_=bin_e[:])
            if with_b_yo:
                b_yo = cst.tile([P, 2], dt.float32, tag="b_yo")
                nc.sync.dma_start(out=b_yo[:], in_=byo_e[:])
            if with_b_head:
                b_hd = cst.tile([1, A], dt.bfloat16, tag="b_hd")
                nc.sync.dma_start(out=b_hd[:], in_=bhd_e[:])

            ident_f = cst.tile([H, H], dt.float32, tag="ident_f")
            make_identity(nc, ident_f[:])
            ident_pb = cst.tile([P, P], dt.bfloat16, tag="ident_pb")
            make_identity(nc, ident_pb[:])
            # causal ones: UT[j,i] = 1 where j<=i (cumsum matmul)
            ut_ones = cst.tile([L, L], dt.float32, tag="ut_ones")
            nc.gpsimd.memset(ut_ones[:], 1.0)
            nc.gpsimd.affine_select(
                out=ut_ones[:],
                in_=ut_ones[:],
                compare_op=OP.is_ge,
                fill=0.0,
                base=0,
                pattern=[[1, L]],
                channel_multiplier=-1,
            )
            ones_row = cst.tile([1, P], dt.float32, tag="ones_row")
            nc.gpsimd.memset(ones_row[:], 1.0)
            # head-pair row selector: ubp = sel2^T @ urow replicates u_he to
            # partitions 0..63 and u_ho to 64..127 in ONE matmul.  Built with
            # two affine_selects (memset can't start at partition 1).
            sel2 = cst.tile([2, P], dt.bfloat16, tag="sel2")
            nc.gpsimd.memset(sel2[:], 1.0)
            nc.gpsimd.affine_select(
                out=sel2[:], in_=sel2[:], compare_op=OP.is_ge, fill=0.0,
                base=0, pattern=[[1, P]], channel_multiplier=-N,
            )
            nc.gpsimd.affine_select(
                out=sel2[:], in_=sel2[:], compare_op=OP.is_ge, fill=0.0,
                base=N - 1, pattern=[[-1, P]], channel_multiplier=N,
            )

            # ---------------- activations / state ------------------
            xT = cst.tile([P, KT * T], dt.bfloat16, tag="xT")  # (d, t)
            x = cst.tile([P, NCH * D], dt.bfloat16, tag="x")  # (t, d) per t-tile
            y = cst.tile([P, KT * T], dt.bfloat16, tag="y")  # (d, t) per d-tile
            zT = obsT  # reuse: obs fully consumed by phase 1
            logit = cst.tile([P, NCH * A], dt.float32, tag="logit")
            # B stacked per head-pair: pair mt at cols [mt*T,(mt+1)*T), rows
            # 0..63 = head 2mt (n), 64..127 = head 2mt+1.  This is exactly the
            # projection psum layout, so evac is a single DVE copy (no DMA).
            bm2 = cst.tile([P, 8 * T], dt.bfloat16, tag="bm2")
            # odd heads' (n,t) at base partition 0 (2-deep manual ring by mt)
            bmho = cst.tile([N, 2 * T], dt.bfloat16, tag="bmho")
            # Cw per head (n, t): head h at cols [h*T,(h+1)*T)
            cw = cst.tile([N, H * T], dt.bfloat16, tag="cw")
            # zero-padded block-diag C for the pair-batched G matmuls.  Manual
            # 2-deep ring by quad: half q%2 at cols [half*2048,(half+1)*2048),
            # layout (pair-in-quad, chunk, head-in-pair, t); zeros persist.
            cm2z = cst.tile([P, 2 * 2048], dt.bfloat16, tag="cm2z")
            nc.gpsimd.memset(cm2z[:], 0.0)
            # B^T (t, n) blocks for the S update.  Ring half per quad,
            # layout (hi, chunk, n): col = half*1024 + hi*256 + c*64 + n.
            bmt2 = cst.tile([P, 2 * 1024], dt.bfloat16, tag="bmt2")
            s_st = [
                cst.tile([N, P], dt.bfloat16, tag=f"s{h}", name=f"s{h}")
                for h in range(H)
            ]
            for h in range(H):
                nc.gpsimd.memset(s_st[h][:], 0.0)

            dtT = cst.tile([H, T], dt.float32, tag="dtT")
            logdtT = cst.tile([H, T], dt.float32, tag="logdtT")
            pcumT = cst.tile([H, T], dt.float32, tag="pcumT")  # Pcum rows (h,t)
            # two-float (hi/lo bf16) splits feeding the K=4 Diff matmuls
            pcumH = cst.tile([H, T], dt.bfloat16, tag="pcumH")
            pcumL = cst.tile([H, T], dt.bfloat16, tag="pcumL")
            npdH = cst.tile([H, T], dt.bfloat16, tag="npdH")
            npdL = cst.tile([H, T], dt.bfloat16, tag="npdL")
            plrow = cst.tile([1, NCH * H], dt.float32, tag="plrow")
            u_all = cst.tile([H, T], dt.bfloat16, tag="u_all")  # exp(Pcum)
            cols = cst.tile([P, NCH * 2 * H], dt.float32, tag="cols")  # [PcumCol|dtCol]
            e2c = cst.tile([P, NCH * H], dt.float32, tag="e2c")  # exp(Plast-Pcum)
            dtotc = cst.tile([P, NCH * H], dt.float32, tag="dtotc")
            ulast = cst.tile([1, NCH * H], dt.float32, tag="ulast")
            # K=4 Diff operand packs for ALL (chunk, head) tiles: cols
            # (c, h, t); group (c,hg) slices a contiguous (4, 4L) window.
            # lh rows [1, npdH, 1, npdL]; rp rows [pcumH, 1, pcumL, 1].
            lh_all = cst.tile([4, NCH * H * L], dt.bfloat16, tag="lh_all")
            rp_all = cst.tile([4, NCH * H * L], dt.bfloat16, tag="rp_all")
            nc.gpsimd.memset(lh_all[:], 1.0)
            nc.gpsimd.memset(rp_all[:], 1.0)

            # ---------------- x^T = relu(W_in^T obs^T) (d,t) --------
            for kt in range(KT):
                ps = ps_proj.tile([P, T], dt.float32, tag="proj")
                for ko in range(2):
                    nc.tensor.matmul(
                        ps[:],
                        w_in[:, ko * D + kt * P : ko * D + (kt + 1) * P],
                        obsT[:, ko * T : (ko + 1) * T],
                        start=(ko == 0),
                        stop=(ko == 1),
                    )
                if with_b_in:
                    nc.scalar.activation(
                        xT[:, kt * T : (kt + 1) * T], ps[:], AF.Relu,
                        bias=b_in[:, kt : kt + 1],
                    )
                else:
                    nc.scalar.activation(xT[:, kt * T : (kt + 1) * T], ps[:], AF.Relu)

            # ---------------- x = xT^T via PE transposes (t,d) ------
            for tt in range(NCH):
                for g4 in range(4):
                    ps = ps_proj.tile([P, 4 * P], dt.bfloat16, tag="proj")
                    for kk in range(4):
                        kt = g4 * 4 + kk
                        nc.tensor.transpose(
                            ps[:, kk * P : (kk + 1) * P],
                            xT[:, kt * T + tt * P : kt * T + (tt + 1) * P],
                            ident_pb[:],
                        )
                    nc.vector.tensor_copy(
                        x[:, tt * D + g4 * 512 : tt * D + (g4 + 1) * 512], ps[:]
                    )

            # ---------------- dt chain ------------------------------
            psd = ps_proj.tile([H, T], dt.float32, tag="proj", name="psd")
            for kt in range(KT):
                nc.tensor.matmul(
                    psd[:],
                    w_dt[:, kt * H : (kt + 1) * H],
                    xT[:, kt * T : (kt + 1) * T],
                    start=(kt == 0),
                    stop=(kt == KT - 1),
                )
            # softplus via ln(1+exp(.)) — Softplus shares no ACT table with
            # Exp/Ln on this compiler; exp/ln/relu/copy live in one table.
            ez = ps_diff.tile([H, T], dt.float32, tag="diff", name="ez")
            nc.scalar.activation(ez[:], psd[:], AF.Exp, bias=dtb[:])
            nc.vector.tensor_scalar_add(ez[:], ez[:], 1.0)
            nc.scalar.activation(dtT[:], ez[:], AF.Ln)
            nc.scalar.activation(logdtT[:], dtT[:], AF.Ln)

            def prep_chunk(c):
                cb = slice(c * L, (c + 1) * L)
                ldec = wrk.tile([H, L], dt.float32, tag="ldec")
                nc.vector.tensor_scalar_mul(ldec[:], dtT[:, cb], neg_a[:])
                pt = ps_tiny.tile([P, 2 * H], dt.float32, tag="tiny")
                nc.tensor.transpose(pt[:, 0:H], ldec[:], ident_f[:])
                ldec_c = wrk.tile([P, H], dt.float32, tag="ldec_c")
                nc.vector.tensor_copy(ldec_c[:], pt[:, 0:H])
                pp = ps_tiny.tile([H, L], dt.float32, tag="tiny")
                nc.tensor.matmul(pp[:], ldec_c[:], ut_ones[:], start=True, stop=True)
                nc.vector.tensor_copy(pcumT[:, cb], pp[:])
                npdc = wrk.tile([H, L], dt.float32, tag="npdc")
                nc.vector.tensor_sub(npdc[:], logdtT[:, cb], pcumT[:, cb])
                nc.vector.tensor_copy(pcumH[:, cb], pcumT[:, cb])
                nc.vector.tensor_sub(pcumL[:, cb], pcumT[:, cb], pcumH[:, cb])
                nc.vector.tensor_copy(npdH[:, cb], npdc[:])
                nc.vector.tensor_sub(npdL[:, cb], npdc[:], npdH[:, cb])
                nc.scalar.activation(u_all[:, cb], pcumT[:, cb], AF.Exp)
                pt2 = ps_tiny.tile([P, 2 * H], dt.float32, tag="tiny")
                nc.tensor.transpose(pt2[:, 0:H], pcumT[:, cb], ident_f[:])
                nc.tensor.transpose(pt2[:, H : 2 * H], dtT[:, cb], ident_f[:])
                co = c * 2 * H
                nc.vector.tensor_copy(cols[:, co : co + 2 * H], pt2[:])
                # PcumLast per head at base partition 0 (row 127 of PcumCol)
                nc.sync.dma_start(
                    out=plrow[:, c * H : (c + 1) * H],
                    in_=cols[L - 1 : L, co : co + H],
                )
                plast = plrow[:, c * H : (c + 1) * H]
                nc.scalar.activation(ulast[:, c * H : (c + 1) * H], plast, AF.Exp)
                pdt = ps_tiny.tile([P, H], dt.float32, tag="tiny")
                nc.tensor.matmul(
                    pdt[:], ones_row[0:1, 0:P], ulast[:, c * H : (c + 1) * H],
                    start=True, stop=True,
                )
                nc.vector.tensor_copy(dtotc[:, c * H : (c + 1) * H], pdt[:])
                ppl = ps_tiny.tile([P, H], dt.float32, tag="tiny")
                nc.tensor.matmul(
                    ppl[:], ones_row[0:1, 0:P], plast, start=True, stop=True
                )
                e2a = wrk.tile([P, H], dt.float32, tag="e2a")
                nc.vector.tensor_sub(e2a[:], ppl[:], cols[:, co : co + H])
                nc.scalar.activation(e2c[:, c * H : (c + 1) * H], e2a[:], AF.Exp)

            # ---------------- B / C projections ---------------------
            # pair j covers mts (2j, 2j+1) = head quad j; one contiguous
            # 256-col weight load per pair.
            wbv = w_b_e.rearrange("(kt p) (j m) -> p kt j m", p=P, j=4)
            wcv = w_c_e.rearrange("(kt p) (j m) -> p kt j m", p=P, j=4)

            def emit_wload(j):
                wbuf2 = wrk.tile([P, KT * 2 * P], dt.bfloat16, tag="wbs", bufs=2)
                nc.sync.dma_start(
                    out=wbuf2[:].rearrange("p (kt m) -> p kt m", kt=KT),
                    in_=wbv[:, :, j, :],
                )
                wcuf2 = wrk.tile([P, KT * 2 * P], dt.bfloat16, tag="wcs", bufs=2)
                nc.scalar.dma_start(
                    out=wcuf2[:].rearrange("p (kt m) -> p kt m", kt=KT),
                    in_=wcv[:, :, j, :],
                )
                return wbuf2, wcuf2

            def emit_projBC(mt, wbuf2, wcuf2, mid=None):
                u = mt % 2
                half = (mt // 2) % 2
                psb = ps_proj.tile([P, T], dt.float32, tag="proj")
                for kt in range(KT):
                    nc.tensor.matmul(
                        psb[:],
                        wbuf2[:, kt * 2 * P + u * P : kt * 2 * P + (u + 1) * P],
                        xT[:, kt * T : (kt + 1) * T],
                        start=(kt == 0),
                        stop=(kt == KT - 1),
                    )
                nc.vector.tensor_copy(bm2[:, mt * T : (mt + 1) * T], psb[:])
                nc.sync.dma_start(
                    out=bmho[:, u * T : (u + 1) * T],
                    in_=bm2[N:P, mt * T : (mt + 1) * T],
                )
                if mid is not None:
                    mid()
                psc = ps_proj.tile([P, T], dt.float32, tag="proj")
                for kt in range(KT):
                    nc.tensor.matmul(
                        psc[:],
                        wcuf2[:, kt * 2 * P + u * P : kt * 2 * P + (u + 1) * P],
                        xT[:, kt * T : (kt + 1) * T],
                        start=(kt == 0),
                        stop=(kt == KT - 1),
                    )
                ctmp = wrk.tile([P, T], dt.bfloat16, tag="ctmp", bufs=2)
                nc.vector.tensor_copy(ctmp[:], psc[:])
                # block-diag C fills (zeros persist in the off-diag blocks)
                vh = cm2z[0:N, half * 2048 : (half + 1) * 2048].rearrange(
                    "n (pp c q t) -> n pp c q t", pp=2, c=NCH, q=2
                )
                nc.sync.dma_start(
                    out=vh[:, u : u + 1, :, 0:1, :],
                    in_=ctmp[0:N, :].rearrange("n (c t) -> n c t", c=NCH),
                )
                vl = cm2z[N:P, half * 2048 : (half + 1) * 2048].rearrange(
                    "n (pp c q t) -> n pp c q t", pp=2, c=NCH, q=2
                )
                nc.scalar.dma_start(
                    out=vl[:, u : u + 1, :, 1:2, :],
                    in_=ctmp[N:P, :].rearrange("n (c t) -> n c t", c=NCH),
                )
                # B^T (t,n) blocks: head 2mt from bm2 rows 0..63, head 2mt+1
                # from the base-0 bmho copy.
                ptr = ps_tiny.tile([P, 4 * P], dt.bfloat16, tag="tiny")
                for q in range(2):
                    for c in range(NCH):
                        if q == 0:
                            src = bm2[0:N, mt * T + c * L : mt * T + (c + 1) * L]
                        else:
                            src = bmho[:, u * T + c * L : u * T + (c + 1) * L]
                        nc.tensor.transpose(
                            ptr[:, q * NCH * N + c * N : q * NCH * N + (c + 1) * N],
                            src,
                            ident_pb[0:N, 0:N],
                        )
                nc.vector.tensor_copy(
                    bmt2[:, half * 1024 + u * 512 : half * 1024 + (u + 1) * 512],
                    ptr[:],
                )
                return ctmp

            def emit_cw(mt, ctmp):
                he, ho = 2 * mt, 2 * mt + 1
                urow = wrk.tile([2, T], dt.bfloat16, tag="urow")
                nc.scalar.dma_start(out=urow[:], in_=u_all[he : ho + 1, :])
                ubp = ps_diff.tile([P, 4 * L], dt.float32, tag="diff", name="ubp")
                nc.tensor.matmul(ubp[:], sel2[:], urow[:], start=True, stop=True)
                ubc = wrk.tile([P, T], dt.bfloat16, tag="ubc", bufs=2)
                nc.scalar.activation(ubc[:], ubp[:], AF.Copy)
                wtmp = wrk.tile([P, T], dt.bfloat16, tag="wtmp", bufs=2)
                nc.vector.tensor_mul(wtmp[:], ctmp[:], ubc[:])
                nc.sync.dma_start(out=cw[:, he * T : (he + 1) * T], in_=wtmp[0:N, :])
                nc.scalar.dma_start(out=cw[:, ho * T : (ho + 1) * T], in_=wtmp[N:P, :])

            # prep interleaved INTO the first pair's projections (between
            # their B and C halves) so the PE never sits in prep's serial
            # transpose/cumsum chain.
            wl = {0: emit_wload(0)}
            prep_chunk(0)
            ct0 = emit_projBC(
                0, *wl[0], mid=lambda: (prep_chunk(1), prep_chunk(2))
            )
            ct1 = emit_projBC(1, *wl[0], mid=lambda: prep_chunk(3))
            wl[1] = emit_wload(1)
            # bulk-stage the Diff packs: lh rows 1/3 <- npd hi/lo, rp rows
            # 0/2 <- pcum hi/lo; one contiguous (16,L) fill per (row, chunk).
            for c in range(NCH):
                cb = slice(c * L, (c + 1) * L)
                for row, src, eng in (
                    (1, npdH, nc.sync),
                    (3, npdL, nc.scalar),
                ):
                    eng.dma_start(
                        out=lh_all[row : row + 1, c * H * L : (c + 1) * H * L]
                        .rearrange("p (h t) -> p h t", h=H),
                        in_=src[:, cb],
                    )
                for row, src, eng in (
                    (0, pcumH, nc.sync),
                    (2, pcumL, nc.scalar),
                ):
                    eng.dma_start(
                        out=rp_all[row : row + 1, c * H * L : (c + 1) * H * L]
                        .rearrange("p (h t) -> p h t", h=H),
                        in_=src[:, cb],
                    )
            emit_cw(0, ct0)
            emit_cw(1, ct1)

            # ---------------- scan ----------------------------------
            yv = y[:].rearrange("p (h t) -> p h t", h=KT)
            # e_sb / gw rings carved from the dead w_in tile (bf16,
            # (P, 4096) = 8 x 512-col slots).  w_in's last read is phase 1.
            esb_ring = [w_in[:, i * 512 : (i + 1) * 512] for i in range(3)]
            gw_ring = [w_in[:, (3 + i) * 512 : (4 + i) * 512] for i in range(3)]
            st = {"pend": [], "g": 0, "z": 0}

            def z_head(h):
                for ut in range(2):
                    nc.tensor.matmul(
                        z_ps[ut][:],
                        w_yo[:, h * U + ut * P : h * U + (ut + 1) * P],
                        y[:, h * T : (h + 1) * T],
                        start=(st["z"] == 0),
                        stop=(st["z"] == H - 1),
                    )
                st["z"] += 1

            def group_front(c, hg):
                half = hg % 2
                dbank = ps_diff.tile([P, 4 * L], dt.float32, tag="diff")
                for hi in range(4):
                    h = 4 * hg + hi
                    sl = slice(c * H * L + h * L, c * H * L + (h + 1) * L)
                    nc.tensor.matmul(
                        dbank[:, hi * L : (hi + 1) * L],
                        lh_all[:, sl],
                        rp_all[:, sl],
                        start=True,
                        stop=True,
                    )
                gbank = ps_gy.tile([P, 4 * L], dt.float32, tag="gy")
                for pq in range(2):
                    mt = 2 * hg + pq
                    nc.tensor.matmul(
                        gbank[:, pq * 2 * L : (pq + 1) * 2 * L],
                        bm2[:, mt * T + c * L : mt * T + (c + 1) * L],
                        cm2z[
                            :,
                            half * 2048 + pq * 1024 + c * 256 : half * 2048
                            + pq * 1024
                            + (c + 1) * 256,
                        ],
                        start=True,
                        stop=True,
                    )
                gi = st["g"]
                e_sb = esb_ring[gi % 3]
                nc.scalar.activation(e_sb, dbank[:], AF.Exp)
                # causal mask: keep i>=j else 0 (kills the exp-overflow infs)
                nc.gpsimd.affine_select(
                    out=e_sb,
                    in_=e_sb,
                    compare_op=OP.is_ge,
                    fill=0.0,
                    base=0,
                    pattern=[[0, 4], [1, L]],
                    channel_multiplier=-1,
                )
                gw = gw_ring[gi % 3]
                nc.vector.tensor_mul(gw, gbank[:], e_sb)
                if len(st["pend"]) >= 2:
                    st["pend"].pop(0)()

                def consume(c=c, hg=hg, gw=gw, half=half):
                    ybank = ps_gy.tile([P, 4 * L], dt.float32, tag="gy")
                    sdb = ps_sd.tile([N, 4 * P], dt.float32, tag="sd")
                    bd = wrk.tile([P, 4 * N], dt.bfloat16, tag="bd")
                    for hi in range(4):
                        h = 4 * hg + hi
                        xc = x[:, c * D + h * P : c * D + (h + 1) * P]
                        nc.tensor.matmul(
                            ybank[:, hi * L : (hi + 1) * L],
                            xc,
                            gw[:, hi * L : (hi + 1) * L],
                            start=True,
                            stop=False,
                        )
                        nc.tensor.matmul(
                            ybank[:, hi * L : (hi + 1) * L],
                            s_st[h][:],
                            cw[:, h * T + c * L : h * T + (c + 1) * L],
                            start=False,
                            stop=True,
                        )
                        nc.vector.tensor_scalar(
                            bd[:, hi * N : (hi + 1) * N],
                            bmt2[
                                :,
                                half * 1024 + hi * 256 + c * N : half * 1024
                                + hi * 256
                                + (c + 1) * N,
                            ],
                            e2c[:, c * H + h : c * H + h + 1],
                            cols[:, c * 2 * H + H + h : c * 2 * H + H + h + 1],
                            op0=OP.mult,
                            op1=OP.mult,
                        )
                        sds = sdb[:, hi * P : (hi + 1) * P]
                        nc.tensor.matmul(
                            sds, bd[:, hi * N : (hi + 1) * N], xc,
                            start=True, stop=True,
                        )
                        nc.vector.scalar_tensor_tensor(
                            s_st[h][:],
                            s_st[h][:],
                            dtotc[0:N, c * H + h : c * H + h + 1],
                            sds,
                            op0=OP.mult,
                            op1=OP.add,
                        )
                    # Y evac: psum (p, (hi,L)) -> y cols (4*hg+hi, c*L..)
                    nc.scalar.activation(
                        yv[:, 4 * hg : 4 * hg + 4, c * L : (c + 1) * L],
                        ybank[:].rearrange("p (i t) -> p i t", i=4),
                        AF.Copy,
                    )

                st["pend"].append(consume)
                st["g"] += 1

            # z emission order for the final round: quad-2 heads must come
            # after consume(3,2), which fires at group (0,3)'s front.
            zq3 = [0, 1, 2, 3, 4, 5, 6, 7, 8, 9, 10, 11]
            for hg in range(4):
                for c in range(NCH):
                    group_front(c, hg)
                    if hg < 3:
                        if c == 1:
                            ct_a = emit_projBC(2 * hg + 2, *wl[hg + 1])
                        elif c == 2:
                            ct_b = emit_projBC(2 * hg + 3, *wl[hg + 1])
                        elif c == 3:
                            emit_cw(2 * hg + 2, ct_a)
                            emit_cw(2 * hg + 3, ct_b)
                            if hg + 2 <= 3:
                                wl[hg + 2] = emit_wload(hg + 2)
                    else:
                        for k in range(3):
                            z_head(zq3[c * 3 + k])
                if hg == 2:
                    z_ps = [
                        ps_proj.tile([P, T], dt.float32, tag="proj", name=f"z{ut}")
                        for ut in range(2)
                    ]
            for pend in st["pend"]:
                pend()
            for h in (12, 13, 14, 15):
                z_head(h)

            # ---------------- z = relu(y W_yo) (u,t) ----------------
            for ut in range(2):
                if with_b_yo:
                    nc.scalar.activation(
                        zT[:, ut * T : (ut + 1) * T], z_ps[ut][:], AF.Relu,
                        bias=b_yo[:, ut : ut + 1],
                    )
                else:
                    nc.scalar.activation(
                        zT[:, ut * T : (ut + 1) * T], z_ps[ut][:], AF.Relu
                    )

            # ---------------- logits --------------------------------
            for tt in range(NCH):
                ps = ps_proj.tile([P, A], dt.float32, tag="proj")
                nmm = 3 if with_b_head else 2
                for ut in range(2):
                    nc.tensor.matmul(
                        ps[:],
                        zT[:, ut * T + tt * P : ut * T + (tt + 1) * P],
                        w_hd[:, ut * A : (ut + 1) * A],
                        start=(ut == 0),
                        stop=(ut == nmm - 1),
                    )
                if with_b_head:
                    nc.tensor.matmul(
                        ps[:],
                        lh_all[0:1, tt * P : (tt + 1) * P],
                        b_hd[:],
                        start=False,
                        stop=True,
                    )
                nc.scalar.activation(logit[:, tt * A : (tt + 1) * A], ps[:], AF.Copy)
                nc.sync.dma_start(
                    out=out_e[tt * P : (tt + 1) * P, :],
                    in_=logit[:, tt * A : (tt + 1) * A],
                )

    _split_multi_waits(nc)
    return nc


def kernel(obs, W_in, b_in, A_log, dt_bias, W_dt, W_B, W_C, W_yo, b_yo, W_head, b_head):
    _inject_axon_hooks()
    _patch_tile()
    from concourse.bass_utils import run_bass_kernel_spmd

    obs = np.asarray(obs, dtype=np.float32)
    flags = (
        bool(np.any(np.asarray(b_in) != 0)),
        bool(np.any(np.asarray(b_yo) != 0)),
        bool(np.any(np.asarray(b_head) != 0)),
    )
    # First call: build once (the verified path). Repeat calls in one
    # process rebuild a fresh graph — re-executing a previously-run nc with
    # new inputs has crashed the exec unit (NRT status 101) in testing.
    if flags not in _CACHE:
        _CACHE[flags] = _build(*flags)
    elif _EXECUTED.get(flags):
        _CACHE[flags] = _build(*flags)
    nc = _CACHE[flags]
    _EXECUTED[flags] = True

    obsT = obs.reshape(T, BSZ, OBSD).transpose(1, 2, 0)  # (B, 256, T)
    base = {
        "w_in": np.ascontiguousarray(W_in).astype(BF16),
        "w_dt": np.ascontiguousarray(W_dt).astype(BF16),
        "w_b": np.ascontiguousarray(W_B).astype(BF16),
        "w_c": np.ascontiguousarray(W_C).astype(BF16),
        "w_yo": np.ascontiguousarray(W_yo).astype(BF16),
        "w_hd": np.ascontiguousarray(W_head).astype(BF16),
        "neg_a": (-np.exp(np.asarray(A_log, np.float64)))
        .astype(np.float32)
        .reshape(H, 1),
        "dtb": np.asarray(dt_bias, np.float32).reshape(H, 1),
    }
    if flags[0]:
        base["b_in"] = np.ascontiguousarray(
            np.asarray(b_in, np.float32).reshape(KT, P).T
        )
    if flags[1]:
        base["b_yo"] = np.ascontiguousarray(
            np.asarray(b_yo, np.float32).reshape(2, P).T
        )
    if flags[2]:
        base["b_hd"] = np.asarray(b_head).astype(BF16).reshape(1, A)
    in_maps = [
        dict(base, obsT=np.ascontiguousarray(obsT[c]).astype(BF16)) for c in range(BSZ)
    ]
    global _last_in_maps
    _last_in_maps = in_maps
    res = run_bass_kernel_spmd(nc, in_maps, core_ids=list(range(BSZ)))
    out = np.stack([res.results[c]["out"] for c in range(BSZ)], axis=1)
    return out.astype(np.float32)


# revision 31
# speedup vs baseline: 1.1874x; 1.0259x over previous
"""Self-contained Trainium2 kernel for the SSD-scan actor network.

Data-parallel over batch B=8 across 8 NeuronCores (one sample per core, no
collectives). Per core:
  x  = relu(obs @ W_in + b_in)                  (T=512, D=2048)
  dt = softplus(x @ W_dt + dt_bias)             (T, H=16)
  Bm = x @ W_B, Cm = x @ W_C                    (T, H, N=64)
  y  = selective scan over T (Mamba2 SSD)       (T, D)
  z  = relu(y @ W_yo + b_yo)                    (T, U=256)
  out = z @ W_head + b_head                     (T, A=64)

The scan uses the chunked (segsum) SSD formulation: chunk length L=128,
4 chunks, 16 independent heads. Per head/chunk:
  E[j,i]  = exp(Pcum_i - Pcum_j + log dt_j), causally masked to j<=i
  Y^T     = x_chunk^T Gw + S_prev^T (C*u),  Gw = (B C^T)^T . E, u_i=exp(Pcum_i)
  S_new   = exp(Pcum_L-1) S_prev + sum_j exp(Pcum_L-1 - Pcum_j) dt_j B_j x_j^T
Big matmuls run in bf16 with fp32 PSUM accumulation; the Diff matrix
(Pcum_i - Pcum_j + logdt_j) is built exactly with K=4 bf16 hi/lo-split
matmuls from operand packs bulk-staged once after the dt chain.

v4 scheduling (the performance-critical part):
  Warm matmuls pipeline at the theoretical array rate (no per-instruction
  overhead), but the PE clock halves (HAM K=4/8) whenever matmul density
  drops for a few microseconds.  So the whole kernel is arranged as one
  dense PE stream: scan head-quads are processed round-by-round with the
  NEXT quad's B/C projections (the dense GEMMs) interleaved between scan
  groups, and the z-projection interleaved into the final round.  The
  head-pair G matmuls are batched via the naturally-stacked projection
  psum layout (bm2 + zero-padded block-diag cm2z), which also removes the
  B-repack DMAs.  All scan operand staging is bulk (16 contiguous fills),
  since each DMA costs ~600ns of issuing-engine time.

Hardware notes (all discovered the hard way on this container's stack):
  - walrus here allows only ONE sync wait per instruction -> _split_multi_waits
  - compute-engine APs (matmul operands, DVE/ACT/gpsimd in or out) must
    start at partition 0 — anything else fails BIR verification or crashes
    the exec unit.  Only DMA may address other partitions, so B/C heads
    are repacked to base-0 tensors via bf16 staging + SBUF-to-SBUF DMA.
  - Softplus shares no ACT function table with Exp/Ln -> ln(1+exp(x)).
  - PSUM pools are bank-granular per tag: keep the baseline 8-bank layout.
"""

import sys
import types

import numpy as np
import ml_dtypes

T, BSZ, OBSD = 512, 8, 256
D, H, N, P = 2048, 16, 64, 128
U, A = 256, 64
L, NCH, KT = 128, 4, 16  # chunk length, #chunks, #d-tiles (D/128)
HN = H * N
BF16 = ml_dtypes.bfloat16

_CACHE = {}
_EXECUTED = {}


def _patch_tile():
    """Split the TileContext final drain's waits across single-wait nops."""
    from concourse import tile, mybir
    from concourse.vector_clock import ScopedClock

    if getattr(tile.TileContext, "_drain_patched", False):
        return

    def _patched(self, tick_clock, wait_clock):
        nc = self.nc
        probe = nc.sync.nop()
        wait_clock.add_sem_waits(
            probe.ins, ScopedClock({None: tick_clock.global_clock})
        )
        si = probe.ins.sync_info
        if si is not None and len(si.on_wait) > 1:
            waits = list(si.on_wait)
            probe.ins.sync_info = mybir.SyncInfo(
                on_wait=[waits[0]], on_update=list(si.on_update)
            )
            for w in waits[1:]:
                nop = nc.sync.nop()
                nop.ins.sync_info = mybir.SyncInfo(on_wait=[w], on_update=[])
        nc.sync.drain()
        nc.all_engine_barrier(sem_only=True)
        assert self.sems is not None
        popped = nc._tile_sem_poison_stack.pop()
        assert popped is self._sem_poison
        nc.clear_and_free_semaphores(list(self.sems.allocated().values()))
        nc.all_engine_barrier(sem_only=True)

    tile.TileContext._drain_and_barrier = _patched
    tile.TileContext._drain_patched = True


def _split_multi_waits(nc):
    """This walrus build accepts at most one sync wait per instruction.
    Hoist extra waits onto single-wait NoOps inserted just before, on the
    same engine (the sequencer stalls there first — strictly conservative)."""
    from concourse import mybir

    n = 0
    for f in nc.m.functions:
        for bb in f.blocks:
            insts = list(bb.instructions)
            changed = False
            new = []
            for inst in insts:
                try:
                    si = inst.sync_info
                except Exception:
                    si = None
                if si is not None and len(si.on_wait) > 1:
                    waits = list(si.on_wait)
                    for w in waits[:-1]:
                        nop = mybir.InstNoOp(
                            name=f"wsplit-{n}", ins=[], outs=[], engine=inst.engine
                        )
                        n += 1
                        nop.sync_info = mybir.SyncInfo(on_wait=[w], on_update=[])
                        nc.register_instruction(nop, overwrite=True)
                        new.append(nop)
                    inst.sync_info = mybir.SyncInfo(
                        on_wait=[waits[-1]], on_update=list(si.on_update)
                    )
                    changed = True
                new.append(inst)
            if changed:
                bb.instructions = new


def _inject_axon_hooks():
    """Make trace=True work (and a BASS_TRACE env var safe) in this container."""
    if "antenv.axon_hooks" not in sys.modules:
        try:
            from trn_agent_boot.trn_boot import _ntff_profile_via_ctypes

            hook = _ntff_profile_via_ctypes("/opt/axon/libaxon_pjrt.so")
        except Exception:
            hook = None
        mod = types.ModuleType("antenv.axon_hooks")
        mod.get_axon_ntff_profile_hook = lambda: hook
        mod.set_axon_ntff_profile_hook = lambda h: None
        sys.modules["antenv.axon_hooks"] = mod
    from concourse import bass_utils

    bass_utils.upload_artifacts = lambda d: d


def _build(with_b_in, with_b_yo, with_b_head):
    import concourse.bass as bass
    import concourse.mybir as mybir
    from concourse.tile import TileContext
    from concourse.masks import make_identity

    dt = mybir.dt
    AF = mybir.ActivationFunctionType
    OP = mybir.AluOpType

    nc = bass.Bass()
    obsT_e = nc.declare_dram_parameter("obsT", [OBSD, T], dt.bfloat16, isOutput=False)
    w_in_e = nc.declare_dram_parameter("w_in", [OBSD, D], dt.bfloat16, isOutput=False)
    w_dt_e = nc.declare_dram_parameter("w_dt", [D, H], dt.bfloat16, isOutput=False)
    w_b_e = nc.declare_dram_parameter("w_b", [D, HN], dt.bfloat16, isOutput=False)
    w_c_e = nc.declare_dram_parameter("w_c", [D, HN], dt.bfloat16, isOutput=False)
    w_yo_e = nc.declare_dram_parameter("w_yo", [D, U], dt.bfloat16, isOutput=False)
    w_hd_e = nc.declare_dram_parameter("w_hd", [U, A], dt.bfloat16, isOutput=False)
    neg_a_e = nc.declare_dram_parameter("neg_a", [H, 1], dt.float32, isOutput=False)
    dtb_e = nc.declare_dram_parameter("dtb", [H, 1], dt.float32, isOutput=False)
    bin_e = byo_e = bhd_e = None
    if with_b_in:
        bin_e = nc.declare_dram_parameter("b_in", [P, KT], dt.float32, isOutput=False)
    if with_b_yo:
        byo_e = nc.declare_dram_parameter("b_yo", [P, 2], dt.float32, isOutput=False)
    if with_b_head:
        bhd_e = nc.declare_dram_parameter("b_hd", [1, A], dt.bfloat16, isOutput=False)
    out_e = nc.declare_dram_parameter("out", [T, A], dt.float32, isOutput=True)

    _patch_tile()
    with TileContext(nc) as tc:
        with (
            tc.tile_pool(name="cst", bufs=1) as cst,
            tc.tile_pool(name="wrk", bufs=2) as wrk,
            tc.tile_pool(name="ps_proj", bufs=2, space="PSUM") as ps_proj,
            tc.tile_pool(name="ps_diff", bufs=2, space="PSUM") as ps_diff,
            tc.tile_pool(name="ps_gy", bufs=2, space="PSUM") as ps_gy,
            tc.tile_pool(name="ps_sd", bufs=1, space="PSUM") as ps_sd,
            tc.tile_pool(name="ps_tiny", bufs=1, space="PSUM") as ps_tiny,
        ):
            # ---------------- weights + constants -----------------
            # obsT/w_in halves split across both HWDGE queues so phase 1
            # starts as early as possible.
            obsT = cst.tile([P, 2 * T], dt.bfloat16, tag="obsT")
            nc.sync.dma_start(out=obsT[:, 0:T], in_=obsT_e[0:P, :])
            nc.scalar.dma_start(out=obsT[:, T : 2 * T], in_=obsT_e[P : 2 * P, :])
            w_in = cst.tile([P, 2 * D], dt.bfloat16, tag="w_in")
            nc.sync.dma_start(out=w_in[:, 0 : D // 2], in_=w_in_e[0:P, 0 : D // 2])
            nc.sync.dma_start(out=w_in[:, D // 2 : D], in_=w_in_e[0:P, D // 2 : D])
            nc.scalar.dma_start(
                out=w_in[:, D : D + D // 2], in_=w_in_e[P : 2 * P, 0 : D // 2]
            )
            nc.scalar.dma_start(
                out=w_in[:, D + D // 2 : 2 * D], in_=w_in_e[P : 2 * P, D // 2 : D]
            )
            w_dt = cst.tile([P, KT * H], dt.bfloat16, tag="w_dt")
            nc.sync.dma_start(
                out=w_dt[:].rearrange("p (k h) -> p k h", k=KT),
                in_=w_dt_e.rearrange("(k p) h -> p k h", p=P),
            )
            w_yo = cst.tile([P, KT * U], dt.bfloat16, tag="w_yo")
            nc.scalar.dma_start(
                out=w_yo[:].rearrange("p (k u) -> p k u", k=KT),
                in_=w_yo_e.rearrange("(k p) u -> p k u", p=P),
            )
            w_hd = cst.tile([P, 2 * A], dt.bfloat16, tag="w_hd")
            nc.scalar.dma_start(
                out=w_hd[:].rearrange("p (k a) -> p k a", k=2),
                in_=w_hd_e.rearrange("(k p) a -> p k a", p=P),
            )
            neg_a = cst.tile([H, 1], dt.float32, tag="neg_a")
            nc.sync.dma_start(out=neg_a[:], in_=neg_a_e[:])
            dtb = cst.tile([H, 1], dt.float32, tag="dtb")
            nc.sync.dma_start(out=dtb[:], in_=dtb_e[:])
            if with_b_in:
                b_in = cst.tile([P, KT], dt.float32, tag="b_in")
                nc.sync.dma_start(out=b_in[:], in_=bin_e[:])
            if with_b_yo:
                b_yo = cst.tile([P, 2], dt.float32, tag="b_yo")
                nc.sync.dma_start(out=b_yo[:], in_=byo_e[:])
            if with_b_head:
                b_hd = cst.tile([1, A], dt.bfloat16, tag="b_hd")
                nc.sync.dma_start(out=b_hd[:], in_=bhd_e[:])

            ident_f = cst.tile([H, H], dt.float32, tag="ident_f")
            make_identity(nc, ident_f[:])
            ident_pb = cst.tile([P, P], dt.bfloat16, tag="ident_pb")
            make_identity(nc, ident_pb[:])
            # causal ones: UT[j,i] = 1 where j<=i (cumsum matmul)
            ut_ones = cst.tile([L, L], dt.float32, tag="ut_ones")
            nc.gpsimd.memset(ut_ones[:], 1.0)
            nc.gpsimd.affine_select(
                out=ut_ones[:],
                in_=ut_ones[:],
                compare_op=OP.is_ge,
                fill=0.0,
                base=0,
                pattern=[[1, L]],
                channel_multiplier=-1,
            )
            ones_row = cst.tile([1, P], dt.float32, tag="ones_row")
            nc.gpsimd.memset(ones_row[:], 1.0)
            # head-pair row selector: ubp = sel2^T @ urow replicates u_he to
            # partitions 0..63 and u_ho to 64..127 in ONE matmul.  Built with
            # two affine_selects (memset can't start at partition 1).
            sel2 = cst.tile([2, P], dt.bfloat16, tag="sel2")
            nc.gpsimd.memset(sel2[:], 1.0)
            nc.gpsimd.affine_select(
                out=sel2[:], in_=sel2[:], compare_op=OP.is_ge, fill=0.0,
                base=0, pattern=[[1, P]], channel_multiplier=-N,
            )
            nc.gpsimd.affine_select(
                out=sel2[:], in_=sel2[:], compare_op=OP.is_ge, fill=0.0,
                base=N - 1, pattern=[[-1, P]], channel_multiplier=N,
            )

            # ---------------- activations / state ------------------
            xT = cst.tile([P, KT * T], dt.bfloat16, tag="xT")  # (d, t)
            x = cst.tile([P, NCH * D], dt.bfloat16, tag="x")  # (t, d) per t-tile
            y = cst.tile([P, KT * T], dt.bfloat16, tag="y")  # (d, t) per d-tile
            zT = obsT  # reuse: obs fully consumed by phase 1
            logit = cst.tile([P, NCH * A], dt.float32, tag="logit")
            # B stacked per head-pair: pair mt at cols [mt*T,(mt+1)*T), rows
            # 0..63 = head 2mt (n), 64..127 = head 2mt+1.  This is exactly the
            # projection psum layout, so evac is a single DVE copy (no DMA).
            bm2 = cst.tile([P, 8 * T], dt.bfloat16, tag="bm2")
            # odd heads' (n,t) at base partition 0 (2-deep manual ring by mt)
            bmho = cst.tile([N, 2 * T], dt.bfloat16, tag="bmho")
            # Cw per head (n, t): head h at cols [h*T,(h+1)*T)
            cw = cst.tile([N, H * T], dt.bfloat16, tag="cw")
            # zero-padded block-diag C for the pair-batched G matmuls.  Manual
            # 2-deep ring by quad: half q%2 at cols [half*2048,(half+1)*2048),
            # layout (pair-in-quad, chunk, head-in-pair, t); zeros persist.
            cm2z = cst.tile([P, 2 * 2048], dt.bfloat16, tag="cm2z")
            nc.gpsimd.memset(cm2z[:], 0.0)
            # B^T (t, n) blocks for the S update.  Ring half per quad,
            # layout (hi, chunk, n): col = half*1024 + hi*256 + c*64 + n.
            bmt2 = cst.tile([P, 2 * 1024], dt.bfloat16, tag="bmt2")
            s_st = [
                cst.tile([N, P], dt.bfloat16, tag=f"s{h}", name=f"s{h}")
                for h in range(H)
            ]
            for h in range(H):
                nc.gpsimd.memset(s_st[h][:], 0.0)

            dtT = cst.tile([H, T], dt.float32, tag="dtT")
            logdtT = cst.tile([H, T], dt.float32, tag="logdtT")
            pcumT = cst.tile([H, T], dt.float32, tag="pcumT")  # Pcum rows (h,t)
            # two-float (hi/lo bf16) splits feeding the K=4 Diff matmuls
            pcumH = cst.tile([H, T], dt.bfloat16, tag="pcumH")
            pcumL = cst.tile([H, T], dt.bfloat16, tag="pcumL")
            npdH = cst.tile([H, T], dt.bfloat16, tag="npdH")
            npdL = cst.tile([H, T], dt.bfloat16, tag="npdL")
            plrow = cst.tile([1, NCH * H], dt.float32, tag="plrow")
            u_all = cst.tile([H, T], dt.bfloat16, tag="u_all")  # exp(Pcum)
            cols = cst.tile([P, NCH * 2 * H], dt.float32, tag="cols")  # [PcumCol|dtCol]
            e2c = cst.tile([P, NCH * H], dt.float32, tag="e2c")  # exp(Plast-Pcum)
            dtotc = cst.tile([P, NCH * H], dt.float32, tag="dtotc")
            ulast = cst.tile([1, NCH * H], dt.float32, tag="ulast")
            # K=4 Diff operand packs for ALL (chunk, head) tiles: cols
            # (c, h, t); group (c,hg) slices a contiguous (4, 4L) window.
            # lh rows [1, npdH, 1, npdL]; rp rows [pcumH, 1, pcumL, 1].
            lh_all = cst.tile([4, NCH * H * L], dt.bfloat16, tag="lh_all")
            rp_all = cst.tile([4, NCH * H * L], dt.bfloat16, tag="rp_all")
            nc.gpsimd.memset(lh_all[:], 1.0)
            nc.gpsimd.memset(rp_all[:], 1.0)

            # ---------------- x^T = relu(W_in^T obs^T) (d,t) --------
            for kt in range(KT):
                ps = ps_proj.tile([P, T], dt.float32, tag="proj")
                for ko in range(2):
                    nc.tensor.matmul(
                        ps[:],
                        w_in[:, ko * D + kt * P : ko * D + (kt + 1) * P],
                        obsT[:, ko * T : (ko + 1) * T],
                        start=(ko == 0),
                        stop=(ko == 1),
                    )
                if with_b_in:
                    nc.scalar.activation(
                        xT[:, kt * T : (kt + 1) * T], ps[:], AF.Relu,
                        bias=b_in[:, kt : kt + 1],
                    )
                else:
                    nc.scalar.activation(xT[:, kt * T : (kt + 1) * T], ps[:], AF.Relu)

            # ---------------- x = xT^T via PE transposes (t,d) ------
            for tt in range(NCH):
                for g4 in range(4):
                    ps = ps_proj.tile([P, 4 * P], dt.bfloat16, tag="proj")
                    for kk in range(4):
                        kt = g4 * 4 + kk
                        nc.tensor.transpose(
                            ps[:, kk * P : (kk + 1) * P],
                            xT[:, kt * T + tt * P : kt * T + (tt + 1) * P],
                            ident_pb[:],
                        )
                    nc.vector.tensor_copy(
                        x[:, tt * D + g4 * 512 : tt * D + (g4 + 1) * 512], ps[:]
                    )

            # ---------------- dt chain ------------------------------
            psd = ps_proj.tile([H, T], dt.float32, tag="proj", name="psd")
            for kt in range(KT):
                nc.tensor.matmul(
                    psd[:],
                    w_dt[:, kt * H : (kt + 1) * H],
                    xT[:, kt * T : (kt + 1) * T],
                    start=(kt == 0),
                    stop=(kt == KT - 1),
                )
            # softplus via ln(1+exp(.)) — Softplus shares no ACT table with
            # Exp/Ln on this compiler; exp/ln/relu/copy live in one table.
            ez = ps_diff.tile([H, T], dt.float32, tag="diff", name="ez")
            nc.scalar.activation(ez[:], psd[:], AF.Exp, bias=dtb[:])
            nc.vector.tensor_scalar_add(ez[:], ez[:], 1.0)
            nc.scalar.activation(dtT[:], ez[:], AF.Ln)
            nc.scalar.activation(logdtT[:], dtT[:], AF.Ln)

            def prep_chunk(c):
                cb = slice(c * L, (c + 1) * L)
                ldec = wrk.tile([H, L], dt.float32, tag="ldec")
                nc.vector.tensor_scalar_mul(ldec[:], dtT[:, cb], neg_a[:])
                pt = ps_tiny.tile([P, 2 * H], dt.float32, tag="tiny")
                nc.tensor.transpose(pt[:, 0:H], ldec[:], ident_f[:])
                ldec_c = wrk.tile([P, H], dt.float32, tag="ldec_c")
                nc.vector.tensor_copy(ldec_c[:], pt[:, 0:H])
                pp = ps_tiny.tile([H, L], dt.float32, tag="tiny")
                nc.tensor.matmul(pp[:], ldec_c[:], ut_ones[:], start=True, stop=True)
                nc.vector.tensor_copy(pcumT[:, cb], pp[:])
                npdc = wrk.tile([H, L], dt.float32, tag="npdc")
                nc.vector.tensor_sub(npdc[:], logdtT[:, cb], pcumT[:, cb])
                nc.vector.tensor_copy(pcumH[:, cb], pcumT[:, cb])
                nc.vector.tensor_sub(pcumL[:, cb], pcumT[:, cb], pcumH[:, cb])
                nc.vector.tensor_copy(npdH[:, cb], npdc[:])
                nc.vector.tensor_sub(npdL[:, cb], npdc[:], npdH[:, cb])
                nc.scalar.activation(u_all[:, cb], pcumT[:, cb], AF.Exp)
                pt2 = ps_tiny.tile([P, 2 * H], dt.float32, tag="tiny")
                nc.tensor.transpose(pt2[:, 0:H], pcumT[:, cb], ident_f[:])
                nc.tensor.transpose(pt2[:, H : 2 * H], dtT[:, cb], ident_f[:])
                co = c * 2 * H
                nc.vector.tensor_copy(cols[:, co : co + 2 * H], pt2[:])
                # PcumLast per head at base partition 0 (row 127 of PcumCol)
                nc.sync.dma_start(
                    out=plrow[:, c * H : (c + 1) * H],
                    in_=cols[L - 1 : L, co : co + H],
                )
                plast = plrow[:, c * H : (c + 1) * H]
                nc.scalar.activation(ulast[:, c * H : (c + 1) * H], plast, AF.Exp)
                pdt = ps_tiny.tile([P, H], dt.float32, tag="tiny")
                nc.tensor.matmul(
                    pdt[:], ones_row[0:1, 0:P], ulast[:, c * H : (c + 1) * H],
                    start=True, stop=True,
                )
                nc.vector.tensor_copy(dtotc[:, c * H : (c + 1) * H], pdt[:])
                ppl = ps_tiny.tile([P, H], dt.float32, tag="tiny")
                nc.tensor.matmul(
                    ppl[:], ones_row[0:1, 0:P], plast, start=True, stop=True
                )
                e2a = wrk.tile([P, H], dt.float32, tag="e2a")
                nc.vector.tensor_sub(e2a[:], ppl[:], cols[:, co : co + H])
                nc.scalar.activation(e2c[:, c * H : (c + 1) * H], e2a[:], AF.Exp)
                # stage this chunk's Diff-pack rows while later work projects
                for row, src, eng in (
                    (1, npdH, nc.sync),
                    (3, npdL, nc.scalar),
                ):
                    eng.dma_start(
                        out=lh_all[row : row + 1, c * H * L : (c + 1) * H * L]
                        .rearrange("p (h t) -> p h t", h=H),
                        in_=src[:, cb],
                    )
                for row, src, eng in (
                    (0, pcumH, nc.sync),
                    (2, pcumL, nc.scalar),
                ):
                    eng.dma_start(
                        out=rp_all[row : row + 1, c * H * L : (c + 1) * H * L]
                        .rearrange("p (h t) -> p h t", h=H),
                        in_=src[:, cb],
                    )

            # ---------------- B / C projections ---------------------
            # pair j covers mts (2j, 2j+1) = head quad j; one contiguous
            # 256-col weight load per pair.
            wbv = w_b_e.rearrange("(kt p) (j m) -> p kt j m", p=P, j=4)
            wcv = w_c_e.rearrange("(kt p) (j m) -> p kt j m", p=P, j=4)

            def emit_wload(j):
                wbuf2 = wrk.tile([P, KT * 2 * P], dt.bfloat16, tag="wbs", bufs=2)
                nc.sync.dma_start(
                    out=wbuf2[:].rearrange("p (kt m) -> p kt m", kt=KT),
                    in_=wbv[:, :, j, :],
                )
                wcuf2 = wrk.tile([P, KT * 2 * P], dt.bfloat16, tag="wcs", bufs=2)
                nc.scalar.dma_start(
                    out=wcuf2[:].rearrange("p (kt m) -> p kt m", kt=KT),
                    in_=wcv[:, :, j, :],
                )
                return wbuf2, wcuf2

            def emit_projBC(mt, wbuf2, wcuf2, mid=None):
                u = mt % 2
                half = (mt // 2) % 2
                psb = ps_proj.tile([P, T], dt.float32, tag="proj")
                for kt in range(KT):
                    nc.tensor.matmul(
                        psb[:],
                        wbuf2[:, kt * 2 * P + u * P : kt * 2 * P + (u + 1) * P],
                        xT[:, kt * T : (kt + 1) * T],
                        start=(kt == 0),
                        stop=(kt == KT - 1),
                    )
                nc.vector.tensor_copy(bm2[:, mt * T : (mt + 1) * T], psb[:])
                nc.sync.dma_start(
                    out=bmho[:, u * T : (u + 1) * T],
                    in_=bm2[N:P, mt * T : (mt + 1) * T],
                )
                if mid is not None:
                    mid()
                psc = ps_proj.tile([P, T], dt.float32, tag="proj")
                for kt in range(KT):
                    nc.tensor.matmul(
                        psc[:],
                        wcuf2[:, kt * 2 * P + u * P : kt * 2 * P + (u + 1) * P],
                        xT[:, kt * T : (kt + 1) * T],
                        start=(kt == 0),
                        stop=(kt == KT - 1),
                    )
                ctmp = wrk.tile([P, T], dt.bfloat16, tag="ctmp", bufs=2)
                nc.vector.tensor_copy(ctmp[:], psc[:])
                # block-diag C fills (zeros persist in the off-diag blocks)
                vh = cm2z[0:N, half * 2048 : (half + 1) * 2048].rearrange(
                    "n (pp c q t) -> n pp c q t", pp=2, c=NCH, q=2
                )
                nc.sync.dma_start(
                    out=vh[:, u : u + 1, :, 0:1, :],
                    in_=ctmp[0:N, :].rearrange("n (c t) -> n c t", c=NCH),
                )
                vl = cm2z[N:P, half * 2048 : (half + 1) * 2048].rearrange(
                    "n (pp c q t) -> n pp c q t", pp=2, c=NCH, q=2
                )
                nc.scalar.dma_start(
                    out=vl[:, u : u + 1, :, 1:2, :],
                    in_=ctmp[N:P, :].rearrange("n (c t) -> n c t", c=NCH),
                )
                # B^T (t,n) blocks: head 2mt from bm2 rows 0..63, head 2mt+1
                # from the base-0 bmho copy.
                ptr = ps_tiny.tile([P, 4 * P], dt.bfloat16, tag="tiny")
                for q in range(2):
                    for c in range(NCH):
                        if q == 0:
                            src = bm2[0:N, mt * T + c * L : mt * T + (c + 1) * L]
                        else:
                            src = bmho[:, u * T + c * L : u * T + (c + 1) * L]
                        nc.tensor.transpose(
                            ptr[:, q * NCH * N + c * N : q * NCH * N + (c + 1) * N],
                            src,
                            ident_pb[0:N, 0:N],
                        )
                # evacuate pre-scaled: bd = B^T * exp(Plast-Pcum) * dt, so the
                # scan's S-update reads bmt2 directly (keeps DVE off the
                # final round's critical path)
                for q in range(2):
                    h = 2 * mt + q
                    hi = 2 * u + q
                    for c in range(NCH):
                        nc.vector.tensor_scalar(
                            bmt2[
                                :,
                                half * 1024 + hi * 256 + c * N : half * 1024
                                + hi * 256
                                + (c + 1) * N,
                            ],
                            ptr[:, q * NCH * N + c * N : q * NCH * N + (c + 1) * N],
                            e2c[:, c * H + h : c * H + h + 1],
                            cols[:, c * 2 * H + H + h : c * 2 * H + H + h + 1],
                            op0=OP.mult,
                            op1=OP.mult,
                        )
                return ctmp

            def emit_cw(mt, ctmp):
                he, ho = 2 * mt, 2 * mt + 1
                urow = wrk.tile([2, T], dt.bfloat16, tag="urow")
                nc.scalar.dma_start(out=urow[:], in_=u_all[he : ho + 1, :])
                ubp = ps_diff.tile([P, 4 * L], dt.float32, tag="diff", name="ubp")
                nc.tensor.matmul(ubp[:], sel2[:], urow[:], start=True, stop=True)
                ubc = wrk.tile([P, T], dt.bfloat16, tag="ubc", bufs=2)
                nc.scalar.activation(ubc[:], ubp[:], AF.Copy)
                wtmp = wrk.tile([P, T], dt.bfloat16, tag="wtmp", bufs=2)
                nc.vector.tensor_mul(wtmp[:], ctmp[:], ubc[:])
                nc.sync.dma_start(out=cw[:, he * T : (he + 1) * T], in_=wtmp[0:N, :])
                nc.scalar.dma_start(out=cw[:, ho * T : (ho + 1) * T], in_=wtmp[N:P, :])

            # prep interleaved INTO the first pair's projections (between
            # their B and C halves) so the PE never sits in prep's serial
            # transpose/cumsum chain.
            wl = {0: emit_wload(0)}
            prep_chunk(0)
            ct0 = emit_projBC(
                0, *wl[0], mid=lambda: (prep_chunk(1), prep_chunk(2))
            )
            ct1 = emit_projBC(1, *wl[0], mid=lambda: prep_chunk(3))
            wl[1] = emit_wload(1)
            emit_cw(0, ct0)
            emit_cw(1, ct1)

            # ---------------- scan ----------------------------------
            yv = y[:].rearrange("p (h t) -> p h t", h=KT)
            # e_sb / gw rings carved from the dead w_in tile (bf16,
            # (P, 4096) = 8 x 512-col slots).  w_in's last read is phase 1.
            esb_ring = [w_in[:, i * 512 : (i + 1) * 512] for i in range(3)]
            gw_ring = [w_in[:, (3 + i) * 512 : (4 + i) * 512] for i in range(3)]
            st = {"pend": [], "g": 0, "z": 0}

            def z_head(h):
                for ut in range(2):
                    nc.tensor.matmul(
                        z_ps[ut][:],
                        w_yo[:, h * U + ut * P : h * U + (ut + 1) * P],
                        y[:, h * T : (h + 1) * T],
                        start=(st["z"] == 0),
                        stop=(st["z"] == H - 1),
                    )
                st["z"] += 1

            def group_front(c, hg):
                half = hg % 2
                dbank = ps_diff.tile([P, 4 * L], dt.float32, tag="diff")
                for hi in range(4):
                    h = 4 * hg + hi
                    sl = slice(c * H * L + h * L, c * H * L + (h + 1) * L)
                    nc.tensor.matmul(
                        dbank[:, hi * L : (hi + 1) * L],
                        lh_all[:, sl],
                        rp_all[:, sl],
                        start=True,
                        stop=True,
                    )
                gbank = ps_gy.tile([P, 4 * L], dt.float32, tag="gy")
                for pq in range(2):
                    mt = 2 * hg + pq
                    nc.tensor.matmul(
                        gbank[:, pq * 2 * L : (pq + 1) * 2 * L],
                        bm2[:, mt * T + c * L : mt * T + (c + 1) * L],
                        cm2z[
                            :,
                            half * 2048 + pq * 1024 + c * 256 : half * 2048
                            + pq * 1024
                            + (c + 1) * 256,
                        ],
                        start=True,
                        stop=True,
                    )
                gi = st["g"]
                e_sb = esb_ring[gi % 3]
                nc.scalar.activation(e_sb, dbank[:], AF.Exp)
                # causal mask: keep i>=j else 0 (kills the exp-overflow infs)
                nc.gpsimd.affine_select(
                    out=e_sb,
                    in_=e_sb,
                    compare_op=OP.is_ge,
                    fill=0.0,
                    base=0,
                    pattern=[[0, 4], [1, L]],
                    channel_multiplier=-1,
                )
                gw = gw_ring[gi % 3]
                nc.vector.tensor_mul(gw, gbank[:], e_sb)
                if len(st["pend"]) >= 2:
                    st["pend"].pop(0)()

                def consume(c=c, hg=hg, gw=gw, half=half):
                    ybank = ps_gy.tile([P, 4 * L], dt.float32, tag="gy")
                    sdb = ps_sd.tile([N, 4 * P], dt.float32, tag="sd")
                    for hi in range(4):
                        h = 4 * hg + hi
                        xc = x[:, c * D + h * P : c * D + (h + 1) * P]
                        nc.tensor.matmul(
                            ybank[:, hi * L : (hi + 1) * L],
                            xc,
                            gw[:, hi * L : (hi + 1) * L],
                            start=True,
                            stop=False,
                        )
                        nc.tensor.matmul(
                            ybank[:, hi * L : (hi + 1) * L],
                            s_st[h][:],
                            cw[:, h * T + c * L : h * T + (c + 1) * L],
                            start=False,
                            stop=True,
                        )
                        sds = sdb[:, hi * P : (hi + 1) * P]
                        nc.tensor.matmul(
                            sds,
                            bmt2[
                                :,
                                half * 1024 + hi * 256 + c * N : half * 1024
                                + hi * 256
                                + (c + 1) * N,
                            ],
                            xc,
                            start=True,
                            stop=True,
                        )
                        nc.vector.scalar_tensor_tensor(
                            s_st[h][:],
                            s_st[h][:],
                            dtotc[0:N, c * H + h : c * H + h + 1],
                            sds,
                            op0=OP.mult,
                            op1=OP.add,
                        )
                    # Y evac: psum (p, (hi,L)) -> y cols (4*hg+hi, c*L..)
                    nc.scalar.activation(
                        yv[:, 4 * hg : 4 * hg + 4, c * L : (c + 1) * L],
                        ybank[:].rearrange("p (i t) -> p i t", i=4),
                        AF.Copy,
                    )

                st["pend"].append(consume)
                st["g"] += 1

            # z emission order for the final round: quad-2 heads must come
            # after consume(3,2), which fires at group (0,3)'s front.
            zq3 = [0, 1, 2, 3, 4, 5, 6, 7, 8, 9, 10, 11]
            for hg in range(4):
                for c in range(NCH):
                    group_front(c, hg)
                    if hg < 3:
                        if c == 1:
                            ct_a = emit_projBC(2 * hg + 2, *wl[hg + 1])
                        elif c == 2:
                            ct_b = emit_projBC(2 * hg + 3, *wl[hg + 1])
                        elif c == 3:
                            emit_cw(2 * hg + 2, ct_a)
                            emit_cw(2 * hg + 3, ct_b)
                            if hg + 2 <= 3:
                                wl[hg + 2] = emit_wload(hg + 2)
                    else:
                        for k in range(3):
                            z_head(zq3[c * 3 + k])
                if hg == 2:
                    z_ps = [
                        ps_proj.tile([P, T], dt.float32, tag="proj", name=f"z{ut}")
                        for ut in range(2)
                    ]
            for pend in st["pend"]:
                pend()
            for h in (12, 13, 14, 15):
                z_head(h)

            # ---------------- z = relu(y W_yo) (u,t) ----------------
            for ut in range(2):
                if with_b_yo:
                    nc.scalar.activation(
                        zT[:, ut * T : (ut + 1) * T], z_ps[ut][:], AF.Relu,
                        bias=b_yo[:, ut : ut + 1],
                    )
                else:
                    nc.scalar.activation(
                        zT[:, ut * T : (ut + 1) * T], z_ps[ut][:], AF.Relu
                    )

            # ---------------- logits --------------------------------
            for tt in range(NCH):
                ps = ps_proj.tile([P, A], dt.float32, tag="proj")
                nmm = 3 if with_b_head else 2
                for ut in range(2):
                    nc.tensor.matmul(
                        ps[:],
                        zT[:, ut * T + tt * P : ut * T + (tt + 1) * P],
                        w_hd[:, ut * A : (ut + 1) * A],
                        start=(ut == 0),
                        stop=(ut == nmm - 1),
                    )
                if with_b_head:
                    nc.tensor.matmul(
                        ps[:],
                        lh_all[0:1, tt * P : (tt + 1) * P],
                        b_hd[:],
                        start=False,
                        stop=True,
                    )
                nc.scalar.activation(logit[:, tt * A : (tt + 1) * A], ps[:], AF.Copy)
                nc.sync.dma_start(
                    out=out_e[tt * P : (tt + 1) * P, :],
                    in_=logit[:, tt * A : (tt + 1) * A],
                )

    _split_multi_waits(nc)
    return nc


def kernel(obs, W_in, b_in, A_log, dt_bias, W_dt, W_B, W_C, W_yo, b_yo, W_head, b_head):
    _inject_axon_hooks()
    _patch_tile()
    from concourse.bass_utils import run_bass_kernel_spmd

    obs = np.asarray(obs, dtype=np.float32)
    flags = (
        bool(np.any(np.asarray(b_in) != 0)),
        bool(np.any(np.asarray(b_yo) != 0)),
        bool(np.any(np.asarray(b_head) != 0)),
    )
    # First call: build once (the verified path). Repeat calls in one
    # process rebuild a fresh graph — re-executing a previously-run nc with
    # new inputs has crashed the exec unit (NRT status 101) in testing.
    if flags not in _CACHE:
        _CACHE[flags] = _build(*flags)
    elif _EXECUTED.get(flags):
        _CACHE[flags] = _build(*flags)
    nc = _CACHE[flags]
    _EXECUTED[flags] = True

    obsT = obs.reshape(T, BSZ, OBSD).transpose(1, 2, 0)  # (B, 256, T)
    base = {
        "w_in": np.ascontiguousarray(W_in).astype(BF16),
        "w_dt": np.ascontiguousarray(W_dt).astype(BF16),
        "w_b": np.ascontiguousarray(W_B).astype(BF16),
        "w_c": np.ascontiguousarray(W_C).astype(BF16),
        "w_yo": np.ascontiguousarray(W_yo).astype(BF16),
        "w_hd": np.ascontiguousarray(W_head).astype(BF16),
        "neg_a": (-np.exp(np.asarray(A_log, np.float64)))
        .astype(np.float32)
        .reshape(H, 1),
        "dtb": np.asarray(dt_bias, np.float32).reshape(H, 1),
    }
    if flags[0]:
        base["b_in"] = np.ascontiguousarray(
            np.asarray(b_in, np.float32).reshape(KT, P).T
        )
    if flags[1]:
        base["b_yo"] = np.ascontiguousarray(
            np.asarray(b_yo, np.float32).reshape(2, P).T
        )
    if flags[2]:
        base["b_hd"] = np.asarray(b_head).astype(BF16).reshape(1, A)
    in_maps = [
        dict(base, obsT=np.ascontiguousarray(obsT[c]).astype(BF16)) for c in range(BSZ)
    ]
    global _last_in_maps
    _last_in_maps = in_maps
    res = run_bass_kernel_spmd(nc, in_maps, core_ids=list(range(BSZ)))
    out = np.stack([res.results[c]["out"] for c in range(BSZ)], axis=1)
    return out.astype(np.float32)
